# revision 59
# baseline (speedup 1.0000x reference)
"""AttentionSubsample Trainium2 kernel.

Full (unsharded) inputs in, full output out. Data-parallel over batch:
32 batches -> 8 NeuronCores x 4 batches each. Weights/biases replicated.

Cost-model 192.4us/core (v1 206.4, v0 275.6). PE busy ~165us (86%) --
every GEMM already at its cheapest dtype mode, so the engine-balance
and pipelining below are what the remaining time is made of:
  - ALL projections (kT, q, v) as fp8(e4m3) hi/lo 3-term DoubleRow
    matmuls (0.5 cyc/row, K=256 packed 2 rows/partition): x=xh+xl,
    W=wh+wl, dropping the lo@lo term. The splits are computed on
    PRE-SCALED tensors (SX/SWK/SWQ/SWV) because the raw weights (~0.06)
    put the lo residuals at fp8-e4m3's subnormal floor (2^-9), which
    wrecks the split (k err 0.78% -> 0.12% scaled). Descale folds into
    the kt evacuation (k/(SX*SWQ) so qt*kt is exact against the
    host-scaled q bias) and the v evacuation.
  - k-channel BN bias dropped entirely: softmax over n is invariant to
    per-q shifts and (k+bk)@q shifts every key n equally.
  - score bias added pre-exp on the PE as fp8 DoubleRow identity
    matmuls: lhsT=(I,0)/(0,I) selects one chunk of an adjacent
    bias-chunk pair; bias resident in SBUF, loaded once.
  - softmax denominators: e-tiles accumulated INCREMENTALLY on DVE
    during the head (no end-of-head serial burst blocking the in-order
    PE SEQ), converted to fp8 (x 2^-6, dodging e4m3's 240 max), summed
    by ONE DoubleRow ones-matmul (the ones live in identp plane 2);
    the 2^-6 reinjects via the po evacuation scale.
  - hswish on Pool (tensor_scalar chains); normalize mult + po/out-proj
    evacuations on DVE; kT/v evacuations split ACT/DVE (Pool cannot
    touch PSUM, scalar_tensor_tensor is not a valid Pool opcode on hw).
  - DMA: hi/lo pairs and bias planes merged into single tensors -- each
    DMA costs ~650ns SP + ~625ns HWDGE serialized regardless of size,
    so transfer COUNT dominates startup. Input pools are double-
    buffered so a batch's DMA never queues behind the previous batch's
    reads on the SP SEQ. PE p-state is warmed with DoubleRow matmuls on
    a memset scratch bridging the ~3us DMA pipe latency.
  - software pipelining: batch b+1's projection psum tiles weave
    between batch b's attention score groups (1/group; batch 0's own v
    chunks weave into its first head at 2/group); the pair output
    projection weaves into the NEXT batch after its projection tiles;
    the final pair runs "deep" (dc 0..6 of three column-tiles first,
    the last-head dc 7 afterwards) to overlap the last hswish chain,
    whose final heads run on DVE's 4x-mode ops instead of Pool.
  - PSUM: scores pool 3x[128,2,512] (chunk pairs at bank-aligned 512
    offsets, exp reads the [*, :320] pair in one ACT instr), po + psm
    1 bank each = 8 banks.
  - error budget (hw-measured rel err 1.33e-2 vs 2e-2 gate): fp8-Z
    rounding ~1e-2, projection splits ~1e-3, bf16 e/vt/t2/wpt ~3e-3.
"""

import sys

if "/opt/trn_rl_repo" not in sys.path:
    sys.path.insert(0, "/opt/trn_rl_repo")

import ml_dtypes
import numpy as np

# --- problem constants (hardcoded, must match the grading reference) ---
B, N, C = 32, 1280, 256
H, KD, D = 8, 64, 128          # heads, key dim, value dim per head
NQ = 320                       # subsampled sequence length
OUT = 384
NCORES = 8
BPC = B // NCORES              # batches per core
EPS = 1e-5
NCH = N // 128                 # 10 n-chunks of 128
GRP = 2                        # scores psum group size (n-chunks per group)
SX, SWK, SWQ, SWV = 16.0, 16.0, 128.0, 16.0   # fp8 hi/lo split pre-scales

_SUB_IDX = np.concatenate([
    (np.arange(32)[::2][:, None] * 32 + np.arange(32)[::2][None, :]).reshape(-1),
    1024 + (np.arange(16)[::2][:, None] * 16 + np.arange(16)[::2][None, :]).reshape(-1),
])  # [320] subsample row gather


def _prep(inputs):
    """Host-side: fold BN into weights, reorder channels, shard over cores."""
    f32 = np.float32
    x = np.asarray(inputs["x"], f32)
    g_kv, b_kv = np.asarray(inputs["g_kv"], f32), np.asarray(inputs["b_kv"], f32)
    rm_kv, rv_kv = np.asarray(inputs["rm_kv"], f32), np.asarray(inputs["rv_kv"], f32)
    g_q, b_q = np.asarray(inputs["g_q"], f32), np.asarray(inputs["b_q"], f32)
    rm_q, rv_q = np.asarray(inputs["rm_q"], f32), np.asarray(inputs["rv_q"], f32)
    g_p, b_p = np.asarray(inputs["g_p"], f32), np.asarray(inputs["b_p"], f32)
    rm_p, rv_p = np.asarray(inputs["rm_p"], f32), np.asarray(inputs["rv_p"], f32)
    W_kv = np.asarray(inputs["W_kv"], f32)
    W_q = np.asarray(inputs["W_q"], f32)
    W_p = np.asarray(inputs["W_p"], f32)
    attn_bias = np.asarray(inputs["attn_bias"], f32)
    bias_idxs = np.asarray(inputs["bias_idxs"])

    s_kv = g_kv / np.sqrt(rv_kv + EPS)
    Wkv_f = W_kv * s_kv[:, None]
    bkv_f = b_kv - rm_kv * s_kv
    kidx = np.concatenate([np.arange(h * 192, h * 192 + KD) for h in range(H)])
    vidx = np.concatenate([np.arange(h * 192 + KD, (h + 1) * 192) for h in range(H)])
    wkt = np.ascontiguousarray(Wkv_f[kidx].T).reshape(2, 128, 512)     # [c,128][512 kch]
    wvt = np.ascontiguousarray(Wkv_f[vidx].T).reshape(2, 128, 1024)
    bvd = np.ascontiguousarray(bkv_f[vidx].reshape(8, 128).T)          # [128, H]

    scale = KD ** -0.5
    s_q = g_q / np.sqrt(rv_q + EPS)
    wqt = np.ascontiguousarray((W_q * (s_q * scale)[:, None]).T).reshape(2, 128, 512)
    # qt carries SX*SWQ*(q+bq); kt carries k/(SX*SWQ) -- product is exact
    bq = np.ascontiguousarray(
        ((b_q - rm_q * s_q) * scale).reshape(4, 128).T) * (SX * SWQ)

    # fp8 hi/lo splits are computed on SCALED tensors: the raw weights
    # (~0.06, and wqt ~0.008 with the attention scale folded) put the lo
    # residuals in fp8-e4m3's subnormal range (floor 2^-9), wrecking the
    # split accuracy (k err 0.78% -> 0.12% with scaling). The product
    # scales are folded into the kt/v evacuations and the q bias.
    f8_ = ml_dtypes.float8_e4m3

    def split8(arr, s):
        a = arr * s
        h = a.astype(f8_)
        l = (a - h.astype(np.float32)).astype(f8_)
        return h, l

    wk8h, wk8l = split8(wkt, SWK)
    wq8h, wq8l = split8(wqt, SWQ)

    s_p = g_p / np.sqrt(rv_p + EPS)
    wpt = np.ascontiguousarray((W_p * s_p[:, None]).T / 6.0).reshape(
        8, 128, OUT).astype(ml_dtypes.bfloat16)
    bps = np.ascontiguousarray(np.broadcast_to(b_p - rm_p * s_p, (128, OUT))).astype(np.float32)

    biasT = attn_bias[:, bias_idxs].transpose(0, 2, 1)                 # [H, N, NQ]
    bias_cpq = biasT.reshape(H, NCH, 128, NQ).transpose(0, 2, 1, 3)    # [H,128,NCH,NQ]
    f8 = ml_dtypes.float8_e4m3
    # bias fp8, stored once per head; the DoubleRow identity pair (I,0)/(0,I)
    # selects one chunk of an adjacent pair per instruction
    bt8 = np.ascontiguousarray(bias_cpq).astype(f8)                    # [H,128,NCH,NQ]

    identp = np.zeros((128, 2, 3, 128), f8)
    identp[np.arange(128), 0, 0, np.arange(128)] = 1.0
    identp[np.arange(128), 1, 1, np.arange(128)] = 1.0
    identp[:, :, 2, :] = 1.0          # ones plane for the fp8 sums matmul

    wv8h, wv8l = split8(wvt, SWV)

    # hi/lo pairs merged into single tensors: each DMA costs ~625ns of
    # serialized HWDGE time regardless of size, so fewer+larger transfers
    # dominate the startup latency
    wk8 = np.stack([wk8h, wk8l])               # [2(hl), 2, 128, 512]
    wq8 = np.stack([wq8h, wq8l])
    wv8 = np.stack([wv8h, wv8l])               # [2(hl), 2, 128, 1024]
    bv2 = np.stack([bvd, bvd + 3.0], axis=1).astype(np.float32)  # [128, 2, H]

    xs = x[:, _SUB_IDX, :]                                             # [B, NQ, C]
    in_maps = []
    for i in range(NCORES):
        sl = slice(i * BPC, (i + 1) * BPC)
        xt = np.ascontiguousarray(x[sl].transpose(0, 2, 1)).reshape(BPC, 2, 128, N)
        x8h, x8l = split8(xt, SX)
        xst = np.ascontiguousarray(xs[sl].transpose(0, 2, 1)).reshape(BPC, 2, 128, NQ)
        xs8h, xs8l = split8(xst, SX)
        in_maps.append({
            "x8": np.ascontiguousarray(np.stack([x8h, x8l], axis=1)),
            "xs8": np.ascontiguousarray(np.stack([xs8h, xs8l], axis=1)),
            "wv8": wv8, "wk8": wk8, "wq8": wq8,
            "wpt": wpt,
            "bq": bq, "bv2": bv2, "bps": bps,
            "bt8": bt8,
            "identp": identp,
        })
    return in_maps


def _body(tc, a, out_ap):
    import concourse.bass as bass  # noqa: F401
    import concourse.mybir as mybir
    from contextlib import ExitStack

    nc = tc.nc
    f32 = mybir.dt.float32
    f32r = mybir.dt.float32r
    bf16 = mybir.dt.bfloat16
    f8e4 = mybir.dt.float8e4
    AF = mybir.ActivationFunctionType
    ALU = mybir.AluOpType
    PM = mybir.MatmulPerfMode
    # kt carries k/(SX*SWQ) so qt (SX*SWQ*(q+bq)) times kt is exact;
    # v psum carries SX*SWV*v
    K_DESCALE = 1.0 / (SX * SWK * SX * SWQ)
    V_DESCALE = 1.0 / (SX * SWV)

    with ExitStack() as ctx:
        ctx.enter_context(
            nc.allow_low_precision(reason="bf16 o-side + fp8 bias matmuls are deliberate; verified vs fp32 reference")
        )
        singles = ctx.enter_context(tc.tile_pool(name="singles", bufs=1))
        # DMA order matters at startup: first-needed weights first (wk8 ->
        # kT projection of batch 0), small attention-phase tiles later.
        # hi/lo fp8 pairs live in one tile (dim1 = hi/lo) = one DMA each.
        wk8 = singles.tile([128, 2, 2, 512], f8e4)
        nc.sync.dma_start(wk8, a["wk8"].rearrange("h c p j -> p h c j"))
        wq8 = singles.tile([128, 2, 2, 512], f8e4)
        bqs = singles.tile([128, 4], f32)
        wv8 = singles.tile([128, 2, 2, 1024], f8e4)
        wp = singles.tile([128, 8, OUT], bf16)
        bv2 = singles.tile([128, 2, H], f32)
        identp = singles.tile([128, 2, 3, 128], f8e4)
        bps = singles.tile([128, OUT], f32)
        bt8a = singles.tile([128, H, NCH, NQ], f8e4)

        # bufs=2: batch b+1's input DMA must not wait on batch b's projection
        # reads -- a bufs=1 ring would stall the DMA on the SP SEQ, blocking
        # every later-issued DMA (bt8, wp) behind it
        x8_p = ctx.enter_context(tc.tile_pool(name="x8", bufs=2))
        xs8_p = ctx.enter_context(tc.tile_pool(name="xs8", bufs=2))
        kt_p = ctx.enter_context(tc.tile_pool(name="kt", bufs=2))
        v_p = ctx.enter_context(tc.tile_pool(name="v", bufs=2))
        qt_p = ctx.enter_context(tc.tile_pool(name="qt", bufs=3))
        e_p = ctx.enter_context(tc.tile_pool(name="e", bufs=10))
        esum_p = ctx.enter_context(tc.tile_pool(name="esum", bufs=3))
        rc_p = ctx.enter_context(tc.tile_pool(name="rc", bufs=3))
        oh_p = ctx.enter_context(tc.tile_pool(name="oh", bufs=3))
        t1_p = ctx.enter_context(tc.tile_pool(name="t1", bufs=3))
        hs_p = ctx.enter_context(tc.tile_pool(name="hs", bufs=2))
        ob_p = ctx.enter_context(tc.tile_pool(name="ob", bufs=4))
        ps_sg = ctx.enter_context(tc.tile_pool(name="ps_sg", bufs=3, space="PSUM"))
        ps_o = ctx.enter_context(tc.tile_pool(name="ps_o", bufs=1, space="PSUM"))
        ps_sum = ctx.enter_context(tc.tile_pool(name="ps_sum", bufs=1, space="PSUM"))

        _wt_n = [0]

        def sg_tile():
            _wt_n[0] += 1
            return ps_sg.tile([128, GRP, 512], f32, tag="sg", name=f"sg{_wt_n[0]}")

        out_flat = out_ap.rearrange("b q o -> (b q) o")

        def dma_x(b, first=False, stagger=False):
            """Issue input DMAs for batch b; returns (x8, xs8).

            x8 data goes FIRST: the weave projection of batch b starts
            consuming it within ~2us of emission, while the staggered
            attention-phase singles (bt8, wp) are not needed until later.
            """
            x8 = x8_p.tile([128, 2, 2, N], f8e4, tag="x8", name=f"x8{b}")
            if first:
                nc.sync.dma_start(
                    x8[:, 0], a["x8"][b, 0].rearrange("c p n -> p c n"))
                nc.sync.dma_start(
                    x8[:, 1], a["x8"][b, 1].rearrange("c p n -> p c n"))
            else:
                nc.sync.dma_start(x8, a["x8"][b].rearrange("h c p n -> p h c n"))
            xs8 = xs8_p.tile([128, 2, 2, NQ], f8e4, tag="xs8", name=f"xs8{b}")
            nc.sync.dma_start(xs8, a["xs8"][b].rearrange("h c p n -> p h c n"))
            if first:
                nc.sync.dma_start(wq8, a["wq8"].rearrange("h c p j -> p h c j"))
                nc.sync.dma_start(bqs, a["bq"])
            if stagger:
                nc.sync.dma_start(
                    bt8a[:, 2:5], a["bt8"][2:5].rearrange("h p c q -> p h c q"))
                nc.sync.dma_start(
                    bt8a[:, 5:8], a["bt8"][5:8].rearrange("h p c q -> p h c q"))
                nc.sync.dma_start(wp, a["wpt"].rearrange("c p j -> p c j"))
                nc.sync.dma_start(bps, a["bps"])
            return x8, xs8

        def proj_gen(b, x8, xs8):
            """Yield after each proj psum tile; returns (kt, vt, qt) eagerly."""
            kt = kt_p.tile([128, 4, N], f32r, tag="kt", name=f"kt{b}")
            vt = v_p.tile([128, NCH, 1024], bf16, tag="vt", name=f"vt{b}")
            qt = qt_p.tile([128, 4, NQ], f32r, tag="qt", name=f"qt{b}")
            kterms = ((0, 0), (1, 0), (0, 1))   # (w hi/lo, x hi/lo) pairs;
            # x-lo only needed by term 3, so batch 0's split x8 DMA overlaps

            def emit():
                # kT projection: fp8 hi/lo 3-term DoubleRow, n-major so the
                # first psum tile only needs DMA slice 0. No bias
                # (softmax-invariant). Evac via 2D-strided copies.
                for ns in range(2):             # n slices 0:512, 512:1024
                    n0 = ns * 512
                    for prh in range(2):        # pr pairs (0,1), (2,3)
                        ps = sg_tile()
                        for j in range(2):
                            pr = 2 * prh + j
                            for pi, (wi, xi) in enumerate(kterms):
                                nc.tensor.matmul(
                                    ps[:, j, :],
                                    lhsT=wk8[:, wi, :, pr * 128:(pr + 1) * 128],
                                    rhs=x8[:, xi, :, n0:n0 + 512],
                                    start=(pi == 0), stop=(pi == 2),
                                    perf_mode=PM.DoubleRow,
                                )
                        if (ns + prh) % 2 == 0:
                            nc.vector.tensor_scalar(
                                kt[:, 2 * prh:2 * prh + 2, n0:n0 + 512], ps,
                                K_DESCALE, None, ALU.mult)
                        else:
                            nc.scalar.activation(
                                kt[:, 2 * prh:2 * prh + 2, n0:n0 + 512], ps,
                                AF.Copy, scale=K_DESCALE)
                        yield
                ps = sg_tile()                  # 256-col tails, two prs per tile
                for prh in range(2):
                    for j in range(2):
                        pr = 2 * prh + j
                        c0 = j * 256
                        for pi, (wi, xi) in enumerate(kterms):
                            nc.tensor.matmul(
                                ps[:, prh, c0:c0 + 256],
                                lhsT=wk8[:, wi, :, pr * 128:(pr + 1) * 128],
                                rhs=x8[:, xi, :, 1024:N],
                                start=(pi == 0), stop=(pi == 2),
                                perf_mode=PM.DoubleRow,
                            )
                    nc.scalar.activation(
                        kt[:, 2 * prh:2 * prh + 2, 1024:N],
                        ps[:, prh, :].rearrange("p (g c) -> p g c", g=2),
                        AF.Copy, scale=K_DESCALE)
                yield
                # q projection: fp8 3-term, 2 prs per tile, fused bias on DVE
                for half in range(2):
                    ps = sg_tile()
                    for j in range(2):
                        pr = 2 * half + j
                        for pi, (wi, xi) in enumerate(kterms):
                            nc.tensor.matmul(
                                ps[:, j, :NQ],
                                lhsT=wq8[:, wi, :, pr * 128:(pr + 1) * 128],
                                rhs=xs8[:, xi, :, :],
                                start=(pi == 0), stop=(pi == 2),
                                perf_mode=PM.DoubleRow,
                            )
                    nc.vector.tensor_tensor(
                        qt[:, 2 * half:2 * half + 2, :], ps[:, :, :NQ],
                        bqs[:, 2 * half:2 * half + 2].to_broadcast((128, 2, NQ)),
                        ALU.add,
                    )
                    yield
                # v projection: fp8 hi/lo DoubleRow (K=256 per pass, 3 passes)
                for cn in range(NCH):
                    ps = sg_tile()
                    for hf in range(2):
                        for pi, (xi, wi) in enumerate(((0, 0), (1, 0), (0, 1))):
                            nc.tensor.matmul(
                                ps[:, hf, :],
                                lhsT=x8[:, xi, :, cn * 128:(cn + 1) * 128],
                                rhs=wv8[:, wi, :, hf * 512:(hf + 1) * 512],
                                start=(pi == 0), stop=(pi == 2),
                                perf_mode=PM.DoubleRow,
                            )
                    if cn % 2 == 0:
                        nc.vector.tensor_scalar(
                            vt[:, cn, :], ps.rearrange("p g j -> p (g j)"),
                            V_DESCALE, None, ALU.mult)
                    else:
                        nc.scalar.activation(
                            vt[:, cn, :], ps.rearrange("p g j -> p (g j)"),
                            AF.Copy, scale=V_DESCALE)
                    yield

            return kt, vt, qt, emit()

        def attention(b, h, kt, vt, qt, t2):
            # generator: yields after each score group so the driver can
            # weave projection tiles of the next batch between groups
            yield
            pr, p0 = h // 2, 64 * (h % 2)
            bt8 = bt8a[:, h]
            po = ps_o.tile([128, NQ], f32, tag="po", name=f"po_{b}_{h}")
            acc = None
            e0 = None
            for g in range(NCH // GRP):
                sg = sg_tile()
                for j in range(GRP):
                    c = GRP * g + j
                    nc.tensor.matmul(
                        sg[:, j, :NQ],
                        lhsT=kt[p0:p0 + 64, pr, c * 128:(c + 1) * 128],
                        rhs=qt[p0:p0 + 64, pr, :],
                        start=True, stop=False,
                    )
                    nc.tensor.matmul(
                        sg[:, j, :NQ],
                        lhsT=identp[:, j, 0:2, :],
                        rhs=bt8[:, GRP * g:GRP * (g + 1), :],
                        start=False, stop=True,
                        perf_mode=PM.DoubleRow,
                    )
                e = e_p.tile([128, GRP, NQ], bf16)
                nc.scalar.activation(e, sg[:, :, :NQ], AF.Exp)
                # incremental e-sum accumulation: spreads the adds through
                # the head so the denominator is ready right after the last
                # exp (no end-of-head serial DVE burst blocking the PE SEQ)
                if g == 0:
                    e0 = e
                elif g == 1:
                    acc = esum_p.tile([128, GRP, NQ], bf16, tag="acc")
                    nc.vector.tensor_tensor(acc, e0, e, ALU.add)
                else:
                    nc.vector.tensor_tensor(acc, acc, e, ALU.add)
                    if g == NCH // GRP - 1:
                        # esum8 = fp8(acc * 2^-6): the scale dodges fp8-e4m3's
                        # 240 max (exp sums reach ~8400); reinjected at oraw
                        esum8 = esum_p.tile([128, GRP, NQ], f8e4, tag="esum8")
                        nc.vector.tensor_scalar(
                            esum8, acc, 2.0 ** -6, None, ALU.mult)
                for j in range(GRP):
                    c = GRP * g + j
                    nc.tensor.matmul(
                        po[:, :NQ],
                        lhsT=vt[:, c, h * 128:(h + 1) * 128],
                        rhs=e[:, j, :],
                        start=(c == 0), stop=(c == NCH - 1),
                    )
                yield
            # softmax denominator: the fold ran right after the last in-head
            # add, so this matmul's input is ready with no serial DVE burst
            psm = ps_sum.tile([128, NQ], f32, tag="psm", name=f"psm_{b}_{h}")
            nc.tensor.matmul(psm, lhsT=identp[:, :, 2, :], rhs=esum8,
                             start=True, stop=True, perf_mode=PM.DoubleRow)
            # copy po out of PSUM immediately (decouples the po bank from the
            # recip chain), then normalize on Pool
            oraw = oh_p.tile([128, NQ], bf16, tag="oraw", name=f"oraw_{b}_{h}")
            nc.vector.tensor_scalar(oraw, po, 2.0 ** -6, None, ALU.mult)
            rc = rc_p.tile([128, NQ], bf16)
            nc.vector.reciprocal(rc, psm)
            oh = oh_p.tile([128, NQ], bf16)
            nc.vector.tensor_tensor(oh, oraw, rc, ALU.mult)
            # hswish: t = min(Relu(o + 3 + bv), 6);  th = (o + bv) * t
            # on Pool normally; the last heads of the last batch run on DVE
            # (143ns/op 4x mode) to shorten the serial chain the output
            # projection tail waits on -- nothing queues behind DVE there
            eng = nc.vector if (b == BPC - 1 and h >= 6) else nc.gpsimd
            t1 = t1_p.tile([128, NQ], bf16)
            eng.tensor_scalar(t1, oh, bv2[:, 1, h:h + 1], 0.0, ALU.add, ALU.max)
            eng.tensor_scalar(t1, t1, 6.0, None, ALU.min)
            obv = t1_p.tile([128, NQ], bf16, tag="obv", name=f"obv_{b}_{h}")
            eng.tensor_scalar(obv, oh, bv2[:, 0, h:h + 1], None, ALU.add)
            eng.tensor_tensor(t2[:, h, b % 2, :], obv, t1, ALU.mult)

        def opj_mm(ps, t2, qc, dc, start, stop):
            nc.tensor.matmul(
                ps[:, 0, :OUT],
                lhsT=t2[:, dc, :, :].rearrange(
                    "p bb q -> p (bb q)")[:, qc * 128:(qc + 1) * 128],
                rhs=wp[:, dc, :],
                start=start, stop=stop,
            )

        def out_proj(b, t2, qcs=range(5), deep=False):
            """deep=True (tail only): run dc 0..6 of three tiles first, dc 7
            (the just-finished last head) last -- overlaps the final head's
            hswish chain with 3.3us of PE work. Holds all 3 sg psum bufs, so
            never weave a deep out_proj into an attention phase."""
            held = []
            if deep:
                for qc in list(qcs)[:3]:
                    ps = sg_tile()
                    for dc in range(7):
                        opj_mm(ps, t2, qc, dc, dc == 0, False)
                    held.append(ps)
            for i, qc in enumerate(qcs):
                r0 = (b - 1) * NQ + qc * 128
                if i < len(held):
                    ps = held[i]
                    opj_mm(ps, t2, qc, 7, False, True)
                else:
                    ps = sg_tile()
                    for dc in range(8):
                        opj_mm(ps, t2, qc, dc, dc == 0, dc == 7)
                ob = ob_p.tile([128, OUT], f32)
                nc.vector.tensor_tensor(ob, ps[:, 0, :OUT], bps, ALU.add)
                nc.sync.dma_start(out_flat[r0:r0 + 128, :], ob)
                yield

        # prologue: batch 0 inputs + weights in order of first use:
        # kT slices -> q inputs -> v weights -> attention singles
        # x8(0) is the largest first-needed transfer -- it leads the
        # serialized DMA pipe; wk8 (small) follows and lands about when
        # the PE warm bridge ends
        x80, xs80 = dma_x(0, first=True)
        nc.sync.dma_start(wv8, a["wv8"].rearrange("h c p j -> p h c j"))
        nc.sync.dma_start(identp, a["identp"])
        nc.sync.dma_start(
            bt8a[:, 0:2], a["bt8"][0:2].rearrange("h p c q -> p h c q"))
        nc.sync.dma_start(bv2, a["bv2"])

        # warm up the PE p-state during the input-DMA wait: dummy matmuls on
        # a memset scratch (no DMA dependency) bridge the ~3.2us DMA pipe
        # latency continuously, so the first real matmul runs at full clock
        scratch = singles.tile([128, 2, 256], f8e4)
        nc.gpsimd.memset(scratch, 0.0)
        warm = sg_tile()
        for i in range(14):
            nc.tensor.matmul(
                warm[:, 0, :256],
                lhsT=scratch[:, :, 0:128],
                rhs=scratch,
                start=True, stop=True,
                perf_mode=PM.DoubleRow,
            )

        kt, vt, qt, gen0 = proj_gen(0, x80, xs80)
        # drain kT + q + v chunks 0,1 eagerly; attention(0) h0 starts ~6us
        # earlier and weaves the remaining v chunks just-in-time (2/group)
        for _ in range(9):
            next(gen0)
        pending = [gen0]
        opj = None
        t2 = None
        nxt = None
        for b in range(BPC):
            if b % 2 == 0:
                t2 = hs_p.tile([128, H, 2, NQ], bf16, tag="t2", name=f"t2_{b}")
            # interleave remaining proj tiles (this batch's tail + next batch)
            if b + 1 < BPC:
                x8n, xs8n = dma_x(b + 1, stagger=(b == 0))
                nxt = proj_gen(b + 1, x8n, xs8n)
                pending.append(nxt[3])
                if opj is not None:
                    # pair out-proj drains AFTER the projection tiles: by
                    # then the previous batch's last-head t2 is long done,
                    # so the weave never stalls PE on the Pool hswish chain
                    pending.append(opj)
                    opj = None

            for h in range(H):
                for gi, _ in enumerate(attention(b, h, kt, vt, qt, t2)):
                    if gi == 0:
                        ndrain = 0
                    elif b == 0 and h == 0:
                        ndrain = 2
                    elif gi == 5:
                        ndrain = 3
                    else:
                        ndrain = 1
                    for _ in range(ndrain):
                        while pending:
                            if next(pending[0], "done") == "done":
                                pending.pop(0)
                            else:
                                break
            while pending:
                if next(pending[0], "done") == "done":
                    pending.pop(0)
                else:
                    break
            if pending:
                for _ in pending[0]:
                    pass
                pending.pop(0)
            if b % 2 == 1:
                if b + 1 < BPC:
                    opj = out_proj(b, t2)  # interleave with next batch
                else:
                    for _ in out_proj(b, t2, deep=True):
                        pass
            if nxt is not None:
                kt, vt, qt = nxt[0], nxt[1], nxt[2]
                nxt = None


def build():
    import concourse.mybir as mybir
    import concourse.tile as tile
    from concourse import bacc

    nc = bacc.Bacc("TRN2", target_bir_lowering=False, debug=False)
    f32, bf16 = mybir.dt.float32, mybir.dt.bfloat16
    f8e4 = mybir.dt.float8e4
    a = {}

    def din(name, shape, dt=f32):
        a[name] = nc.dram_tensor(name, shape, dt, kind="ExternalInput").ap()

    din("x8", [BPC, 2, 2, 128, N], f8e4)
    din("xs8", [BPC, 2, 2, 128, NQ], f8e4)
    din("wk8", [2, 2, 128, 512], f8e4)
    din("wq8", [2, 2, 128, 512], f8e4)
    din("wv8", [2, 2, 128, 1024], f8e4)
    din("wpt", [8, 128, OUT], bf16)
    din("bq", [128, 4])
    din("bv2", [128, 2, H])
    din("bps", [128, OUT])
    din("bt8", [H, 128, NCH, NQ], f8e4)
    din("identp", [128, 2, 3, 128], f8e4)
    out_ap = nc.dram_tensor("out", [BPC, NQ, OUT], f32, kind="ExternalOutput").ap()

    with tile.TileContext(nc) as tc:
        _body(tc, a, out_ap)
    nc.compile()
    return nc


_NC_CACHE = None


def _get_nc():
    global _NC_CACHE
    if _NC_CACHE is None:
        _NC_CACHE = build()
    return _NC_CACHE


def kernel(**inputs):
    from concourse.bass_utils import run_bass_kernel_spmd

    in_maps = _prep(inputs)
    nc = _get_nc()
    res = run_bass_kernel_spmd(nc, in_maps, list(range(NCORES)))
    out = np.concatenate([res.results[i]["out"] for i in range(NCORES)], axis=0)
    return np.ascontiguousarray(out, dtype=np.float32)


if __name__ == "__main__":
    rng = np.random.default_rng(0)
    print("smoke: building bass module...")
    nc = build()
    print("built ok:", sum(len(bb.instructions) for bb in nc.m.functions[0].blocks), "instructions")



# revision 60
# speedup vs baseline: 1.0008x; 1.0008x over previous
"""AttentionSubsample Trainium2 kernel.

Full (unsharded) inputs in, full output out. Data-parallel over batch:
32 batches -> 8 NeuronCores x 4 batches each. Weights/biases replicated.

Cost-model 192.4us/core (v1 206.4, v0 275.6). PE busy ~165us (86%) --
every GEMM already at its cheapest dtype mode, so the engine-balance
and pipelining below are what the remaining time is made of:
  - ALL projections (kT, q, v) as fp8(e4m3) hi/lo 3-term DoubleRow
    matmuls (0.5 cyc/row, K=256 packed 2 rows/partition): x=xh+xl,
    W=wh+wl, dropping the lo@lo term. The splits are computed on
    PRE-SCALED tensors (SX/SWK/SWQ/SWV) because the raw weights (~0.06)
    put the lo residuals at fp8-e4m3's subnormal floor (2^-9), which
    wrecks the split (k err 0.78% -> 0.12% scaled). Descale folds into
    the kt evacuation (k/(SX*SWQ) so qt*kt is exact against the
    host-scaled q bias) and the v evacuation.
  - k-channel BN bias dropped entirely: softmax over n is invariant to
    per-q shifts and (k+bk)@q shifts every key n equally.
  - score bias added pre-exp on the PE as fp8 DoubleRow identity
    matmuls: lhsT=(I,0)/(0,I) selects one chunk of an adjacent
    bias-chunk pair; bias resident in SBUF, loaded once.
  - softmax denominators: e-tiles accumulated INCREMENTALLY on DVE
    during the head (no end-of-head serial burst blocking the in-order
    PE SEQ), converted to fp8 (x 2^-6, dodging e4m3's 240 max), summed
    by ONE DoubleRow ones-matmul (the ones live in identp plane 2);
    the 2^-6 reinjects via the po evacuation scale.
  - hswish on Pool (tensor_scalar chains); normalize mult + po/out-proj
    evacuations on DVE; kT/v evacuations split ACT/DVE (Pool cannot
    touch PSUM, scalar_tensor_tensor is not a valid Pool opcode on hw).
  - DMA: hi/lo pairs and bias planes merged into single tensors -- each
    DMA costs ~650ns SP + ~625ns HWDGE serialized regardless of size,
    so transfer COUNT dominates startup. Input pools are double-
    buffered so a batch's DMA never queues behind the previous batch's
    reads on the SP SEQ. PE p-state is warmed with DoubleRow matmuls on
    a memset scratch bridging the ~3us DMA pipe latency.
  - software pipelining: batch b+1's projection psum tiles weave
    between batch b's attention score groups (1/group; batch 0's own v
    chunks weave into its first head at 2/group); the pair output
    projection weaves into the NEXT batch after its projection tiles;
    the final pair runs "deep" (dc 0..6 of three column-tiles first,
    the last-head dc 7 afterwards) to overlap the last hswish chain,
    whose final heads run on DVE's 4x-mode ops instead of Pool.
  - PSUM: scores pool 3x[128,2,512] (chunk pairs at bank-aligned 512
    offsets, exp reads the [*, :320] pair in one ACT instr), po + psm
    1 bank each = 8 banks.
  - error budget (hw-measured rel err 1.33e-2 vs 2e-2 gate): fp8-Z
    rounding ~1e-2, projection splits ~1e-3, bf16 e/vt/t2/wpt ~3e-3.
"""

import sys

if "/opt/trn_rl_repo" not in sys.path:
    sys.path.insert(0, "/opt/trn_rl_repo")

import ml_dtypes
import numpy as np

# --- problem constants (hardcoded, must match the grading reference) ---
B, N, C = 32, 1280, 256
H, KD, D = 8, 64, 128          # heads, key dim, value dim per head
NQ = 320                       # subsampled sequence length
OUT = 384
NCORES = 8
BPC = B // NCORES              # batches per core
EPS = 1e-5
NCH = N // 128                 # 10 n-chunks of 128
GRP = 2                        # scores psum group size (n-chunks per group)
SX, SWK, SWQ, SWV = 16.0, 16.0, 128.0, 16.0   # fp8 hi/lo split pre-scales

_SUB_IDX = np.concatenate([
    (np.arange(32)[::2][:, None] * 32 + np.arange(32)[::2][None, :]).reshape(-1),
    1024 + (np.arange(16)[::2][:, None] * 16 + np.arange(16)[::2][None, :]).reshape(-1),
])  # [320] subsample row gather


def _prep(inputs):
    """Host-side: fold BN into weights, reorder channels, shard over cores."""
    f32 = np.float32
    x = np.asarray(inputs["x"], f32)
    g_kv, b_kv = np.asarray(inputs["g_kv"], f32), np.asarray(inputs["b_kv"], f32)
    rm_kv, rv_kv = np.asarray(inputs["rm_kv"], f32), np.asarray(inputs["rv_kv"], f32)
    g_q, b_q = np.asarray(inputs["g_q"], f32), np.asarray(inputs["b_q"], f32)
    rm_q, rv_q = np.asarray(inputs["rm_q"], f32), np.asarray(inputs["rv_q"], f32)
    g_p, b_p = np.asarray(inputs["g_p"], f32), np.asarray(inputs["b_p"], f32)
    rm_p, rv_p = np.asarray(inputs["rm_p"], f32), np.asarray(inputs["rv_p"], f32)
    W_kv = np.asarray(inputs["W_kv"], f32)
    W_q = np.asarray(inputs["W_q"], f32)
    W_p = np.asarray(inputs["W_p"], f32)
    attn_bias = np.asarray(inputs["attn_bias"], f32)
    bias_idxs = np.asarray(inputs["bias_idxs"])

    s_kv = g_kv / np.sqrt(rv_kv + EPS)
    Wkv_f = W_kv * s_kv[:, None]
    bkv_f = b_kv - rm_kv * s_kv
    kidx = np.concatenate([np.arange(h * 192, h * 192 + KD) for h in range(H)])
    vidx = np.concatenate([np.arange(h * 192 + KD, (h + 1) * 192) for h in range(H)])
    wkt = np.ascontiguousarray(Wkv_f[kidx].T).reshape(2, 128, 512)     # [c,128][512 kch]
    wvt = np.ascontiguousarray(Wkv_f[vidx].T).reshape(2, 128, 1024)
    bvd = np.ascontiguousarray(bkv_f[vidx].reshape(8, 128).T)          # [128, H]

    scale = KD ** -0.5
    s_q = g_q / np.sqrt(rv_q + EPS)
    wqt = np.ascontiguousarray((W_q * (s_q * scale)[:, None]).T).reshape(2, 128, 512)
    # qt carries SX*SWQ*(q+bq); kt carries k/(SX*SWQ) -- product is exact
    bq = np.ascontiguousarray(
        ((b_q - rm_q * s_q) * scale).reshape(4, 128).T) * (SX * SWQ)

    # fp8 hi/lo splits are computed on SCALED tensors: the raw weights
    # (~0.06, and wqt ~0.008 with the attention scale folded) put the lo
    # residuals in fp8-e4m3's subnormal range (floor 2^-9), wrecking the
    # split accuracy (k err 0.78% -> 0.12% with scaling). The product
    # scales are folded into the kt/v evacuations and the q bias.
    f8_ = ml_dtypes.float8_e4m3

    def split8(arr, s):
        a = arr * s
        h = a.astype(f8_)
        l = (a - h.astype(np.float32)).astype(f8_)
        return h, l

    wk8h, wk8l = split8(wkt, SWK)
    wq8h, wq8l = split8(wqt, SWQ)

    s_p = g_p / np.sqrt(rv_p + EPS)
    wpt = np.ascontiguousarray((W_p * s_p[:, None]).T / 6.0).reshape(
        8, 128, OUT).astype(ml_dtypes.bfloat16)
    bps = np.ascontiguousarray(np.broadcast_to(b_p - rm_p * s_p, (128, OUT))).astype(np.float32)

    biasT = attn_bias[:, bias_idxs].transpose(0, 2, 1)                 # [H, N, NQ]
    bias_cpq = biasT.reshape(H, NCH, 128, NQ).transpose(0, 2, 1, 3)    # [H,128,NCH,NQ]
    f8 = ml_dtypes.float8_e4m3
    # bias fp8, stored once per head; the DoubleRow identity pair (I,0)/(0,I)
    # selects one chunk of an adjacent pair per instruction
    bt8 = np.ascontiguousarray(bias_cpq).astype(f8)                    # [H,128,NCH,NQ]

    identp = np.zeros((128, 2, 3, 128), f8)
    identp[np.arange(128), 0, 0, np.arange(128)] = 1.0
    identp[np.arange(128), 1, 1, np.arange(128)] = 1.0
    identp[:, :, 2, :] = 1.0          # ones plane for the fp8 sums matmul

    wv8h, wv8l = split8(wvt, SWV)

    # hi/lo pairs merged into single tensors: each DMA costs ~625ns of
    # serialized HWDGE time regardless of size, so fewer+larger transfers
    # dominate the startup latency
    wk8 = np.stack([wk8h, wk8l])               # [2(hl), 2, 128, 512]
    wq8 = np.stack([wq8h, wq8l])
    wv8 = np.stack([wv8h, wv8l])               # [2(hl), 2, 128, 1024]
    bv2 = np.stack([bvd, bvd + 3.0], axis=1).astype(np.float32)  # [128, 2, H]

    xs = x[:, _SUB_IDX, :]                                             # [B, NQ, C]
    in_maps = []
    for i in range(NCORES):
        sl = slice(i * BPC, (i + 1) * BPC)
        xt = np.ascontiguousarray(x[sl].transpose(0, 2, 1)).reshape(BPC, 2, 128, N)
        x8h, x8l = split8(xt, SX)
        xst = np.ascontiguousarray(xs[sl].transpose(0, 2, 1)).reshape(BPC, 2, 128, NQ)
        xs8h, xs8l = split8(xst, SX)
        in_maps.append({
            "x8": np.ascontiguousarray(np.stack([x8h, x8l], axis=1)),
            "xs8": np.ascontiguousarray(np.stack([xs8h, xs8l], axis=1)),
            "wv8": wv8, "wk8": wk8, "wq8": wq8,
            "wpt": wpt,
            "bq": bq, "bv2": bv2, "bps": bps,
            "bt8": bt8,
            "identp": identp,
        })
    return in_maps


def _body(tc, a, out_ap):
    import concourse.bass as bass  # noqa: F401
    import concourse.mybir as mybir
    from contextlib import ExitStack

    nc = tc.nc
    f32 = mybir.dt.float32
    f32r = mybir.dt.float32r
    bf16 = mybir.dt.bfloat16
    f8e4 = mybir.dt.float8e4
    AF = mybir.ActivationFunctionType
    ALU = mybir.AluOpType
    PM = mybir.MatmulPerfMode
    # kt carries k/(SX*SWQ) so qt (SX*SWQ*(q+bq)) times kt is exact;
    # v psum carries SX*SWV*v
    K_DESCALE = 1.0 / (SX * SWK * SX * SWQ)
    V_DESCALE = 1.0 / (SX * SWV)

    with ExitStack() as ctx:
        ctx.enter_context(
            nc.allow_low_precision(reason="bf16 o-side + fp8 bias matmuls are deliberate; verified vs fp32 reference")
        )
        singles = ctx.enter_context(tc.tile_pool(name="singles", bufs=1))
        # DMA order matters at startup: first-needed weights first (wk8 ->
        # kT projection of batch 0), small attention-phase tiles later.
        # hi/lo fp8 pairs live in one tile (dim1 = hi/lo) = one DMA each.
        wk8 = singles.tile([128, 2, 2, 512], f8e4)
        nc.sync.dma_start(wk8, a["wk8"].rearrange("h c p j -> p h c j"))
        wq8 = singles.tile([128, 2, 2, 512], f8e4)
        bqs = singles.tile([128, 4], f32)
        wv8 = singles.tile([128, 2, 2, 1024], f8e4)
        wp = singles.tile([128, 8, OUT], bf16)
        bv2 = singles.tile([128, 2, H], f32)
        identp = singles.tile([128, 2, 3, 128], f8e4)
        bps = singles.tile([128, OUT], f32)
        bt8a = singles.tile([128, H, NCH, NQ], f8e4)

        # bufs=2: batch b+1's input DMA must not wait on batch b's projection
        # reads -- a bufs=1 ring would stall the DMA on the SP SEQ, blocking
        # every later-issued DMA (bt8, wp) behind it
        x8_p = ctx.enter_context(tc.tile_pool(name="x8", bufs=2))
        xs8_p = ctx.enter_context(tc.tile_pool(name="xs8", bufs=2))
        kt_p = ctx.enter_context(tc.tile_pool(name="kt", bufs=2))
        v_p = ctx.enter_context(tc.tile_pool(name="v", bufs=2))
        qt_p = ctx.enter_context(tc.tile_pool(name="qt", bufs=3))
        e_p = ctx.enter_context(tc.tile_pool(name="e", bufs=12))
        esum_p = ctx.enter_context(tc.tile_pool(name="esum", bufs=3))
        rc_p = ctx.enter_context(tc.tile_pool(name="rc", bufs=3))
        oh_p = ctx.enter_context(tc.tile_pool(name="oh", bufs=3))
        t1_p = ctx.enter_context(tc.tile_pool(name="t1", bufs=3))
        hs_p = ctx.enter_context(tc.tile_pool(name="hs", bufs=2))
        ob_p = ctx.enter_context(tc.tile_pool(name="ob", bufs=4))
        ps_sg = ctx.enter_context(tc.tile_pool(name="ps_sg", bufs=3, space="PSUM"))
        ps_o = ctx.enter_context(tc.tile_pool(name="ps_o", bufs=1, space="PSUM"))
        ps_sum = ctx.enter_context(tc.tile_pool(name="ps_sum", bufs=1, space="PSUM"))

        _wt_n = [0]

        def sg_tile():
            _wt_n[0] += 1
            return ps_sg.tile([128, GRP, 512], f32, tag="sg", name=f"sg{_wt_n[0]}")

        out_flat = out_ap.rearrange("b q o -> (b q) o")

        def dma_x(b, first=False, stagger=False):
            """Issue input DMAs for batch b; returns (x8, xs8).

            x8 data goes FIRST: the weave projection of batch b starts
            consuming it within ~2us of emission, while the staggered
            attention-phase singles (bt8, wp) are not needed until later.
            """
            x8 = x8_p.tile([128, 2, 2, N], f8e4, tag="x8", name=f"x8{b}")
            if first:
                nc.sync.dma_start(
                    x8[:, 0], a["x8"][b, 0].rearrange("c p n -> p c n"))
                nc.sync.dma_start(
                    x8[:, 1], a["x8"][b, 1].rearrange("c p n -> p c n"))
            else:
                nc.sync.dma_start(x8, a["x8"][b].rearrange("h c p n -> p h c n"))
            xs8 = xs8_p.tile([128, 2, 2, NQ], f8e4, tag="xs8", name=f"xs8{b}")
            nc.sync.dma_start(xs8, a["xs8"][b].rearrange("h c p n -> p h c n"))
            if first:
                nc.sync.dma_start(wq8, a["wq8"].rearrange("h c p j -> p h c j"))
                nc.sync.dma_start(bqs, a["bq"])
            if stagger:
                nc.sync.dma_start(
                    bt8a[:, 2:5], a["bt8"][2:5].rearrange("h p c q -> p h c q"))
                nc.sync.dma_start(
                    bt8a[:, 5:8], a["bt8"][5:8].rearrange("h p c q -> p h c q"))
                nc.sync.dma_start(wp, a["wpt"].rearrange("c p j -> p c j"))
                nc.sync.dma_start(bps, a["bps"])
            return x8, xs8

        def proj_gen(b, x8, xs8):
            """Yield after each proj psum tile; returns (kt, vt, qt) eagerly."""
            kt = kt_p.tile([128, 4, N], f32r, tag="kt", name=f"kt{b}")
            vt = v_p.tile([128, NCH, 1024], bf16, tag="vt", name=f"vt{b}")
            qt = qt_p.tile([128, 4, NQ], f32r, tag="qt", name=f"qt{b}")
            kterms = ((0, 0), (1, 0), (0, 1))   # (w hi/lo, x hi/lo) pairs;
            # x-lo only needed by term 3, so batch 0's split x8 DMA overlaps

            def emit():
                # kT projection: fp8 hi/lo 3-term DoubleRow, n-major so the
                # first psum tile only needs DMA slice 0. No bias
                # (softmax-invariant). Evac via 2D-strided copies.
                for ns in range(2):             # n slices 0:512, 512:1024
                    n0 = ns * 512
                    for prh in range(2):        # pr pairs (0,1), (2,3)
                        ps = sg_tile()
                        for j in range(2):
                            pr = 2 * prh + j
                            for pi, (wi, xi) in enumerate(kterms):
                                nc.tensor.matmul(
                                    ps[:, j, :],
                                    lhsT=wk8[:, wi, :, pr * 128:(pr + 1) * 128],
                                    rhs=x8[:, xi, :, n0:n0 + 512],
                                    start=(pi == 0), stop=(pi == 2),
                                    perf_mode=PM.DoubleRow,
                                )
                        if (ns + prh) % 2 == 0:
                            nc.vector.tensor_scalar(
                                kt[:, 2 * prh:2 * prh + 2, n0:n0 + 512], ps,
                                K_DESCALE, None, ALU.mult)
                        else:
                            nc.scalar.activation(
                                kt[:, 2 * prh:2 * prh + 2, n0:n0 + 512], ps,
                                AF.Copy, scale=K_DESCALE)
                        yield
                ps = sg_tile()                  # 256-col tails, two prs per tile
                for prh in range(2):
                    for j in range(2):
                        pr = 2 * prh + j
                        c0 = j * 256
                        for pi, (wi, xi) in enumerate(kterms):
                            nc.tensor.matmul(
                                ps[:, prh, c0:c0 + 256],
                                lhsT=wk8[:, wi, :, pr * 128:(pr + 1) * 128],
                                rhs=x8[:, xi, :, 1024:N],
                                start=(pi == 0), stop=(pi == 2),
                                perf_mode=PM.DoubleRow,
                            )
                    nc.scalar.activation(
                        kt[:, 2 * prh:2 * prh + 2, 1024:N],
                        ps[:, prh, :].rearrange("p (g c) -> p g c", g=2),
                        AF.Copy, scale=K_DESCALE)
                yield
                # q projection: fp8 3-term, 2 prs per tile, fused bias on DVE
                for half in range(2):
                    ps = sg_tile()
                    for j in range(2):
                        pr = 2 * half + j
                        for pi, (wi, xi) in enumerate(kterms):
                            nc.tensor.matmul(
                                ps[:, j, :NQ],
                                lhsT=wq8[:, wi, :, pr * 128:(pr + 1) * 128],
                                rhs=xs8[:, xi, :, :],
                                start=(pi == 0), stop=(pi == 2),
                                perf_mode=PM.DoubleRow,
                            )
                    nc.vector.tensor_tensor(
                        qt[:, 2 * half:2 * half + 2, :], ps[:, :, :NQ],
                        bqs[:, 2 * half:2 * half + 2].to_broadcast((128, 2, NQ)),
                        ALU.add,
                    )
                    yield
                # v projection: fp8 hi/lo DoubleRow (K=256 per pass, 3 passes)
                for cn in range(NCH):
                    ps = sg_tile()
                    for hf in range(2):
                        for pi, (xi, wi) in enumerate(((0, 0), (1, 0), (0, 1))):
                            nc.tensor.matmul(
                                ps[:, hf, :],
                                lhsT=x8[:, xi, :, cn * 128:(cn + 1) * 128],
                                rhs=wv8[:, wi, :, hf * 512:(hf + 1) * 512],
                                start=(pi == 0), stop=(pi == 2),
                                perf_mode=PM.DoubleRow,
                            )
                    if cn % 2 == 0:
                        nc.vector.tensor_scalar(
                            vt[:, cn, :], ps.rearrange("p g j -> p (g j)"),
                            V_DESCALE, None, ALU.mult)
                    else:
                        nc.scalar.activation(
                            vt[:, cn, :], ps.rearrange("p g j -> p (g j)"),
                            AF.Copy, scale=V_DESCALE)
                    yield

            return kt, vt, qt, emit()

        def attention(b, h, kt, vt, qt, t2):
            # generator: yields after each score group so the driver can
            # weave projection tiles of the next batch between groups
            yield
            pr, p0 = h // 2, 64 * (h % 2)
            bt8 = bt8a[:, h]
            po = ps_o.tile([128, NQ], f32, tag="po", name=f"po_{b}_{h}")
            acc = None
            e0 = None
            for g in range(NCH // GRP):
                sg = sg_tile()
                for j in range(GRP):
                    c = GRP * g + j
                    nc.tensor.matmul(
                        sg[:, j, :NQ],
                        lhsT=kt[p0:p0 + 64, pr, c * 128:(c + 1) * 128],
                        rhs=qt[p0:p0 + 64, pr, :],
                        start=True, stop=False,
                    )
                    nc.tensor.matmul(
                        sg[:, j, :NQ],
                        lhsT=identp[:, j, 0:2, :],
                        rhs=bt8[:, GRP * g:GRP * (g + 1), :],
                        start=False, stop=True,
                        perf_mode=PM.DoubleRow,
                    )
                e = e_p.tile([128, GRP, NQ], bf16)
                nc.scalar.activation(e, sg[:, :, :NQ], AF.Exp)
                # incremental e-sum accumulation: spreads the adds through
                # the head so the denominator is ready right after the last
                # exp (no end-of-head serial DVE burst blocking the PE SEQ)
                if g == 0:
                    e0 = e
                elif g == 1:
                    acc = esum_p.tile([128, GRP, NQ], bf16, tag="acc")
                    nc.vector.tensor_tensor(acc, e0, e, ALU.add)
                else:
                    nc.vector.tensor_tensor(acc, acc, e, ALU.add)
                    if g == NCH // GRP - 1:
                        # esum8 = fp8(acc * 2^-6): the scale dodges fp8-e4m3's
                        # 240 max (exp sums reach ~8400); reinjected at oraw
                        esum8 = esum_p.tile([128, GRP, NQ], f8e4, tag="esum8")
                        nc.vector.tensor_scalar(
                            esum8, acc, 2.0 ** -6, None, ALU.mult)
                for j in range(GRP):
                    c = GRP * g + j
                    nc.tensor.matmul(
                        po[:, :NQ],
                        lhsT=vt[:, c, h * 128:(h + 1) * 128],
                        rhs=e[:, j, :],
                        start=(c == 0), stop=(c == NCH - 1),
                    )
                yield
            # softmax denominator: the fold ran right after the last in-head
            # add, so this matmul's input is ready with no serial DVE burst
            psm = ps_sum.tile([128, NQ], f32, tag="psm", name=f"psm_{b}_{h}")
            nc.tensor.matmul(psm, lhsT=identp[:, :, 2, :], rhs=esum8,
                             start=True, stop=True, perf_mode=PM.DoubleRow)
            # copy po out of PSUM immediately (decouples the po bank from the
            # recip chain), then normalize on Pool
            oraw = oh_p.tile([128, NQ], bf16, tag="oraw", name=f"oraw_{b}_{h}")
            nc.vector.tensor_scalar(oraw, po, 2.0 ** -6, None, ALU.mult)
            rc = rc_p.tile([128, NQ], bf16)
            nc.vector.reciprocal(rc, psm)
            oh = oh_p.tile([128, NQ], bf16)
            nc.vector.tensor_tensor(oh, oraw, rc, ALU.mult)
            # hswish: t = min(Relu(o + 3 + bv), 6);  th = (o + bv) * t
            # on Pool normally; the last heads of the last batch run on DVE
            # (143ns/op 4x mode) to shorten the serial chain the output
            # projection tail waits on -- nothing queues behind DVE there
            eng = nc.vector if (b == BPC - 1 and h >= 6) else nc.gpsimd
            t1 = t1_p.tile([128, NQ], bf16)
            eng.tensor_scalar(t1, oh, bv2[:, 1, h:h + 1], 0.0, ALU.add, ALU.max)
            eng.tensor_scalar(t1, t1, 6.0, None, ALU.min)
            obv = t1_p.tile([128, NQ], bf16, tag="obv", name=f"obv_{b}_{h}")
            eng.tensor_scalar(obv, oh, bv2[:, 0, h:h + 1], None, ALU.add)
            eng.tensor_tensor(t2[:, h, b % 2, :], obv, t1, ALU.mult)

        def opj_mm(ps, t2, qc, dc, start, stop):
            nc.tensor.matmul(
                ps[:, 0, :OUT],
                lhsT=t2[:, dc, :, :].rearrange(
                    "p bb q -> p (bb q)")[:, qc * 128:(qc + 1) * 128],
                rhs=wp[:, dc, :],
                start=start, stop=stop,
            )

        def out_proj(b, t2, qcs=range(5), deep=False):
            """deep=True (tail only): run dc 0..6 of three tiles first, dc 7
            (the just-finished last head) last -- overlaps the final head's
            hswish chain with 3.3us of PE work. Holds all 3 sg psum bufs, so
            never weave a deep out_proj into an attention phase."""
            held = []
            if deep:
                for qc in list(qcs)[:3]:
                    ps = sg_tile()
                    for dc in range(7):
                        opj_mm(ps, t2, qc, dc, dc == 0, False)
                    held.append(ps)
            for i, qc in enumerate(qcs):
                r0 = (b - 1) * NQ + qc * 128
                if i < len(held):
                    ps = held[i]
                    opj_mm(ps, t2, qc, 7, False, True)
                else:
                    ps = sg_tile()
                    for dc in range(8):
                        opj_mm(ps, t2, qc, dc, dc == 0, dc == 7)
                ob = ob_p.tile([128, OUT], f32)
                nc.vector.tensor_tensor(ob, ps[:, 0, :OUT], bps, ALU.add)
                nc.sync.dma_start(out_flat[r0:r0 + 128, :], ob)
                yield

        # prologue: batch 0 inputs + weights in order of first use:
        # kT slices -> q inputs -> v weights -> attention singles
        # x8(0) is the largest first-needed transfer -- it leads the
        # serialized DMA pipe; wk8 (small) follows and lands about when
        # the PE warm bridge ends
        x80, xs80 = dma_x(0, first=True)
        nc.sync.dma_start(wv8, a["wv8"].rearrange("h c p j -> p h c j"))
        nc.sync.dma_start(identp, a["identp"])
        nc.sync.dma_start(
            bt8a[:, 0:2], a["bt8"][0:2].rearrange("h p c q -> p h c q"))
        nc.sync.dma_start(bv2, a["bv2"])

        # warm up the PE p-state during the input-DMA wait: dummy matmuls on
        # a memset scratch (no DMA dependency) bridge the ~3.2us DMA pipe
        # latency continuously, so the first real matmul runs at full clock
        scratch = singles.tile([128, 2, 256], f8e4)
        nc.gpsimd.memset(scratch, 0.0)
        warm = sg_tile()
        for i in range(14):
            nc.tensor.matmul(
                warm[:, 0, :256],
                lhsT=scratch[:, :, 0:128],
                rhs=scratch,
                start=True, stop=True,
                perf_mode=PM.DoubleRow,
            )

        kt, vt, qt, gen0 = proj_gen(0, x80, xs80)
        # drain kT + q + v chunks 0,1 eagerly; attention(0) h0 starts ~6us
        # earlier and weaves the remaining v chunks just-in-time (2/group)
        for _ in range(9):
            next(gen0)
        pending = [gen0]
        opj = None
        t2 = None
        nxt = None
        for b in range(BPC):
            if b % 2 == 0:
                t2 = hs_p.tile([128, H, 2, NQ], bf16, tag="t2", name=f"t2_{b}")
            # interleave remaining proj tiles (this batch's tail + next batch)
            if b + 1 < BPC:
                x8n, xs8n = dma_x(b + 1, stagger=(b == 0))
                nxt = proj_gen(b + 1, x8n, xs8n)
                pending.append(nxt[3])
                if opj is not None:
                    # pair out-proj drains AFTER the projection tiles: by
                    # then the previous batch's last-head t2 is long done,
                    # so the weave never stalls PE on the Pool hswish chain
                    pending.append(opj)
                    opj = None

            for h in range(H):
                for gi, _ in enumerate(attention(b, h, kt, vt, qt, t2)):
                    if gi == 0:
                        ndrain = 0
                    elif b == 0 and h == 0:
                        ndrain = 2
                    elif gi == 5:
                        ndrain = 3
                    else:
                        ndrain = 1
                    for _ in range(ndrain):
                        while pending:
                            if next(pending[0], "done") == "done":
                                pending.pop(0)
                            else:
                                break
            while pending:
                if next(pending[0], "done") == "done":
                    pending.pop(0)
                else:
                    break
            if pending:
                for _ in pending[0]:
                    pass
                pending.pop(0)
            if b % 2 == 1:
                if b + 1 < BPC:
                    opj = out_proj(b, t2)  # interleave with next batch
                else:
                    for _ in out_proj(b, t2, deep=True):
                        pass
            if nxt is not None:
                kt, vt, qt = nxt[0], nxt[1], nxt[2]
                nxt = None


def build():
    import concourse.mybir as mybir
    import concourse.tile as tile
    from concourse import bacc

    nc = bacc.Bacc("TRN2", target_bir_lowering=False, debug=False)
    f32, bf16 = mybir.dt.float32, mybir.dt.bfloat16
    f8e4 = mybir.dt.float8e4
    a = {}

    def din(name, shape, dt=f32):
        a[name] = nc.dram_tensor(name, shape, dt, kind="ExternalInput").ap()

    din("x8", [BPC, 2, 2, 128, N], f8e4)
    din("xs8", [BPC, 2, 2, 128, NQ], f8e4)
    din("wk8", [2, 2, 128, 512], f8e4)
    din("wq8", [2, 2, 128, 512], f8e4)
    din("wv8", [2, 2, 128, 1024], f8e4)
    din("wpt", [8, 128, OUT], bf16)
    din("bq", [128, 4])
    din("bv2", [128, 2, H])
    din("bps", [128, OUT])
    din("bt8", [H, 128, NCH, NQ], f8e4)
    din("identp", [128, 2, 3, 128], f8e4)
    out_ap = nc.dram_tensor("out", [BPC, NQ, OUT], f32, kind="ExternalOutput").ap()

    with tile.TileContext(nc) as tc:
        _body(tc, a, out_ap)
    nc.compile()
    return nc


_NC_CACHE = None


def _get_nc():
    global _NC_CACHE
    if _NC_CACHE is None:
        _NC_CACHE = build()
    return _NC_CACHE


def kernel(**inputs):
    from concourse.bass_utils import run_bass_kernel_spmd

    in_maps = _prep(inputs)
    nc = _get_nc()
    res = run_bass_kernel_spmd(nc, in_maps, list(range(NCORES)))
    out = np.concatenate([res.results[i]["out"] for i in range(NCORES)], axis=0)
    return np.ascontiguousarray(out, dtype=np.float32)


if __name__ == "__main__":
    rng = np.random.default_rng(0)
    print("smoke: building bass module...")
    nc = build()
    print("built ok:", sum(len(bb.instructions) for bb in nc.m.functions[0].blocks), "instructions")



# revision 64
# speedup vs baseline: 1.0010x; 1.0002x over previous
"""AttentionSubsample Trainium2 kernel.

Full (unsharded) inputs in, full output out. Data-parallel over batch:
32 batches -> 8 NeuronCores x 4 batches each. Weights/biases replicated.

Cost-model 192.4us/core (v1 206.4, v0 275.6). PE busy ~165us (86%) --
every GEMM already at its cheapest dtype mode, so the engine-balance
and pipelining below are what the remaining time is made of:
  - ALL projections (kT, q, v) as fp8(e4m3) hi/lo 3-term DoubleRow
    matmuls (0.5 cyc/row, K=256 packed 2 rows/partition): x=xh+xl,
    W=wh+wl, dropping the lo@lo term. The splits are computed on
    PRE-SCALED tensors (SX/SWK/SWQ/SWV) because the raw weights (~0.06)
    put the lo residuals at fp8-e4m3's subnormal floor (2^-9), which
    wrecks the split (k err 0.78% -> 0.12% scaled). Descale folds into
    the kt evacuation (k/(SX*SWQ) so qt*kt is exact against the
    host-scaled q bias) and the v evacuation.
  - k-channel BN bias dropped entirely: softmax over n is invariant to
    per-q shifts and (k+bk)@q shifts every key n equally.
  - score bias added pre-exp on the PE as fp8 DoubleRow identity
    matmuls: lhsT=(I,0)/(0,I) selects one chunk of an adjacent
    bias-chunk pair; bias resident in SBUF, loaded once.
  - softmax denominators: e-tiles accumulated INCREMENTALLY on DVE
    during the head (no end-of-head serial burst blocking the in-order
    PE SEQ), converted to fp8 (x 2^-6, dodging e4m3's 240 max), summed
    by ONE DoubleRow ones-matmul (the ones live in identp plane 2);
    the 2^-6 reinjects via the po evacuation scale.
  - hswish on Pool (tensor_scalar chains); normalize mult + po/out-proj
    evacuations on DVE; kT/v evacuations split ACT/DVE (Pool cannot
    touch PSUM, scalar_tensor_tensor is not a valid Pool opcode on hw).
  - DMA: hi/lo pairs and bias planes merged into single tensors -- each
    DMA costs ~650ns SP + ~625ns HWDGE serialized regardless of size,
    so transfer COUNT dominates startup. Input pools are double-
    buffered so a batch's DMA never queues behind the previous batch's
    reads on the SP SEQ. PE p-state is warmed with DoubleRow matmuls on
    a memset scratch bridging the ~3us DMA pipe latency.
  - software pipelining: batch b+1's projection psum tiles weave
    between batch b's attention score groups (1/group; batch 0's own v
    chunks weave into its first head at 2/group); the pair output
    projection weaves into the NEXT batch after its projection tiles;
    the final pair runs "deep" (dc 0..6 of three column-tiles first,
    the last-head dc 7 afterwards) to overlap the last hswish chain,
    whose final heads run on DVE's 4x-mode ops instead of Pool.
  - PSUM: scores pool 3x[128,2,512] (chunk pairs at bank-aligned 512
    offsets, exp reads the [*, :320] pair in one ACT instr), po + psm
    1 bank each = 8 banks.
  - error budget (hw-measured rel err 1.33e-2 vs 2e-2 gate): fp8-Z
    rounding ~1e-2, projection splits ~1e-3, bf16 e/vt/t2/wpt ~3e-3.
"""

import sys

if "/opt/trn_rl_repo" not in sys.path:
    sys.path.insert(0, "/opt/trn_rl_repo")

import ml_dtypes
import numpy as np

# --- problem constants (hardcoded, must match the grading reference) ---
B, N, C = 32, 1280, 256
H, KD, D = 8, 64, 128          # heads, key dim, value dim per head
NQ = 320                       # subsampled sequence length
OUT = 384
NCORES = 8
BPC = B // NCORES              # batches per core
EPS = 1e-5
NCH = N // 128                 # 10 n-chunks of 128
GRP = 2                        # scores psum group size (n-chunks per group)
SX, SWK, SWQ, SWV = 16.0, 16.0, 128.0, 16.0   # fp8 hi/lo split pre-scales

_SUB_IDX = np.concatenate([
    (np.arange(32)[::2][:, None] * 32 + np.arange(32)[::2][None, :]).reshape(-1),
    1024 + (np.arange(16)[::2][:, None] * 16 + np.arange(16)[::2][None, :]).reshape(-1),
])  # [320] subsample row gather


def _prep(inputs):
    """Host-side: fold BN into weights, reorder channels, shard over cores."""
    f32 = np.float32
    x = np.asarray(inputs["x"], f32)
    g_kv, b_kv = np.asarray(inputs["g_kv"], f32), np.asarray(inputs["b_kv"], f32)
    rm_kv, rv_kv = np.asarray(inputs["rm_kv"], f32), np.asarray(inputs["rv_kv"], f32)
    g_q, b_q = np.asarray(inputs["g_q"], f32), np.asarray(inputs["b_q"], f32)
    rm_q, rv_q = np.asarray(inputs["rm_q"], f32), np.asarray(inputs["rv_q"], f32)
    g_p, b_p = np.asarray(inputs["g_p"], f32), np.asarray(inputs["b_p"], f32)
    rm_p, rv_p = np.asarray(inputs["rm_p"], f32), np.asarray(inputs["rv_p"], f32)
    W_kv = np.asarray(inputs["W_kv"], f32)
    W_q = np.asarray(inputs["W_q"], f32)
    W_p = np.asarray(inputs["W_p"], f32)
    attn_bias = np.asarray(inputs["attn_bias"], f32)
    bias_idxs = np.asarray(inputs["bias_idxs"])

    s_kv = g_kv / np.sqrt(rv_kv + EPS)
    Wkv_f = W_kv * s_kv[:, None]
    bkv_f = b_kv - rm_kv * s_kv
    kidx = np.concatenate([np.arange(h * 192, h * 192 + KD) for h in range(H)])
    vidx = np.concatenate([np.arange(h * 192 + KD, (h + 1) * 192) for h in range(H)])
    wkt = np.ascontiguousarray(Wkv_f[kidx].T).reshape(2, 128, 512)     # [c,128][512 kch]
    wvt = np.ascontiguousarray(Wkv_f[vidx].T).reshape(2, 128, 1024)
    bvd = np.ascontiguousarray(bkv_f[vidx].reshape(8, 128).T)          # [128, H]

    scale = KD ** -0.5
    s_q = g_q / np.sqrt(rv_q + EPS)
    wqt = np.ascontiguousarray((W_q * (s_q * scale)[:, None]).T).reshape(2, 128, 512)
    # qt carries SX*SWQ*(q+bq); kt carries k/(SX*SWQ) -- product is exact
    bq = np.ascontiguousarray(
        ((b_q - rm_q * s_q) * scale).reshape(4, 128).T) * (SX * SWQ)

    # fp8 hi/lo splits are computed on SCALED tensors: the raw weights
    # (~0.06, and wqt ~0.008 with the attention scale folded) put the lo
    # residuals in fp8-e4m3's subnormal range (floor 2^-9), wrecking the
    # split accuracy (k err 0.78% -> 0.12% with scaling). The product
    # scales are folded into the kt/v evacuations and the q bias.
    f8_ = ml_dtypes.float8_e4m3

    def split8(arr, s):
        a = arr * s
        h = a.astype(f8_)
        l = (a - h.astype(np.float32)).astype(f8_)
        return h, l

    wk8h, wk8l = split8(wkt, SWK)
    wq8h, wq8l = split8(wqt, SWQ)

    s_p = g_p / np.sqrt(rv_p + EPS)
    wpt = np.ascontiguousarray((W_p * s_p[:, None]).T / 6.0).reshape(
        8, 128, OUT).astype(ml_dtypes.bfloat16)
    bps = np.ascontiguousarray(np.broadcast_to(b_p - rm_p * s_p, (128, OUT))).astype(np.float32)

    biasT = attn_bias[:, bias_idxs].transpose(0, 2, 1)                 # [H, N, NQ]
    bias_cpq = biasT.reshape(H, NCH, 128, NQ).transpose(0, 2, 1, 3)    # [H,128,NCH,NQ]
    f8 = ml_dtypes.float8_e4m3
    # bias fp8, stored once per head; the DoubleRow identity pair (I,0)/(0,I)
    # selects one chunk of an adjacent pair per instruction
    bt8 = np.ascontiguousarray(bias_cpq).astype(f8)                    # [H,128,NCH,NQ]

    identp = np.zeros((128, 2, 3, 128), f8)
    identp[np.arange(128), 0, 0, np.arange(128)] = 1.0
    identp[np.arange(128), 1, 1, np.arange(128)] = 1.0
    identp[:, :, 2, :] = 1.0          # ones plane for the fp8 sums matmul

    wv8h, wv8l = split8(wvt, SWV)

    # hi/lo pairs merged into single tensors: each DMA costs ~625ns of
    # serialized HWDGE time regardless of size, so fewer+larger transfers
    # dominate the startup latency
    wk8 = np.stack([wk8h, wk8l])               # [2(hl), 2, 128, 512]
    wq8 = np.stack([wq8h, wq8l])
    wv8 = np.stack([wv8h, wv8l])               # [2(hl), 2, 128, 1024]
    bv2 = np.stack([bvd, bvd + 3.0], axis=1).astype(np.float32)  # [128, 2, H]

    xs = x[:, _SUB_IDX, :]                                             # [B, NQ, C]
    in_maps = []
    for i in range(NCORES):
        sl = slice(i * BPC, (i + 1) * BPC)
        xt = np.ascontiguousarray(x[sl].transpose(0, 2, 1)).reshape(BPC, 2, 128, N)
        x8h, x8l = split8(xt, SX)
        xst = np.ascontiguousarray(xs[sl].transpose(0, 2, 1)).reshape(BPC, 2, 128, NQ)
        xs8h, xs8l = split8(xst, SX)
        in_maps.append({
            "x8": np.ascontiguousarray(np.stack([x8h, x8l], axis=1)),
            "xs8": np.ascontiguousarray(np.stack([xs8h, xs8l], axis=1)),
            "wv8": wv8, "wk8": wk8, "wq8": wq8,
            "wpt": wpt,
            "bq": bq, "bv2": bv2, "bps": bps,
            "bt8": bt8,
            "identp": identp,
        })
    return in_maps


def _body(tc, a, out_ap):
    import concourse.bass as bass  # noqa: F401
    import concourse.mybir as mybir
    from contextlib import ExitStack

    nc = tc.nc
    f32 = mybir.dt.float32
    f32r = mybir.dt.float32r
    bf16 = mybir.dt.bfloat16
    f8e4 = mybir.dt.float8e4
    AF = mybir.ActivationFunctionType
    ALU = mybir.AluOpType
    PM = mybir.MatmulPerfMode
    # kt carries k/(SX*SWQ) so qt (SX*SWQ*(q+bq)) times kt is exact;
    # v psum carries SX*SWV*v
    K_DESCALE = 1.0 / (SX * SWK * SX * SWQ)
    V_DESCALE = 1.0 / (SX * SWV)

    with ExitStack() as ctx:
        ctx.enter_context(
            nc.allow_low_precision(reason="bf16 o-side + fp8 bias matmuls are deliberate; verified vs fp32 reference")
        )
        singles = ctx.enter_context(tc.tile_pool(name="singles", bufs=1))
        # DMA order matters at startup: first-needed weights first (wk8 ->
        # kT projection of batch 0), small attention-phase tiles later.
        # hi/lo fp8 pairs live in one tile (dim1 = hi/lo) = one DMA each.
        wk8 = singles.tile([128, 2, 2, 512], f8e4)
        nc.sync.dma_start(wk8, a["wk8"].rearrange("h c p j -> p h c j"))
        wq8 = singles.tile([128, 2, 2, 512], f8e4)
        bqs = singles.tile([128, 4], f32)
        wv8 = singles.tile([128, 2, 2, 1024], f8e4)
        wp = singles.tile([128, 8, OUT], bf16)
        bv2 = singles.tile([128, 2, H], f32)
        identp = singles.tile([128, 2, 3, 128], f8e4)
        bps = singles.tile([128, OUT], f32)
        bt8a = singles.tile([128, H, NCH, NQ], f8e4)

        # bufs=2: batch b+1's input DMA must not wait on batch b's projection
        # reads -- a bufs=1 ring would stall the DMA on the SP SEQ, blocking
        # every later-issued DMA (bt8, wp) behind it
        x8_p = ctx.enter_context(tc.tile_pool(name="x8", bufs=2))
        xs8_p = ctx.enter_context(tc.tile_pool(name="xs8", bufs=2))
        kt_p = ctx.enter_context(tc.tile_pool(name="kt", bufs=2))
        v_p = ctx.enter_context(tc.tile_pool(name="v", bufs=2))
        qt_p = ctx.enter_context(tc.tile_pool(name="qt", bufs=3))
        e_p = ctx.enter_context(tc.tile_pool(name="e", bufs=12))
        esum_p = ctx.enter_context(tc.tile_pool(name="esum", bufs=3))
        rc_p = ctx.enter_context(tc.tile_pool(name="rc", bufs=3))
        oh_p = ctx.enter_context(tc.tile_pool(name="oh", bufs=3))
        t1_p = ctx.enter_context(tc.tile_pool(name="t1", bufs=3))
        hs_p = ctx.enter_context(tc.tile_pool(name="hs", bufs=2))
        ob_p = ctx.enter_context(tc.tile_pool(name="ob", bufs=4))
        ps_sg = ctx.enter_context(tc.tile_pool(name="ps_sg", bufs=3, space="PSUM"))
        ps_o = ctx.enter_context(tc.tile_pool(name="ps_o", bufs=1, space="PSUM"))
        ps_sum = ctx.enter_context(tc.tile_pool(name="ps_sum", bufs=1, space="PSUM"))

        _wt_n = [0]

        def sg_tile():
            _wt_n[0] += 1
            return ps_sg.tile([128, GRP, 512], f32, tag="sg", name=f"sg{_wt_n[0]}")

        out_flat = out_ap.rearrange("b q o -> (b q) o")

        def dma_x(b, first=False, stagger=False):
            """Issue input DMAs for batch b; returns (x8, xs8).

            x8 data goes FIRST: the weave projection of batch b starts
            consuming it within ~2us of emission, while the staggered
            attention-phase singles (bt8, wp) are not needed until later.
            """
            x8 = x8_p.tile([128, 2, 2, N], f8e4, tag="x8", name=f"x8{b}")
            if first:
                nc.sync.dma_start(
                    x8[:, 0], a["x8"][b, 0].rearrange("c p n -> p c n"))
                nc.sync.dma_start(
                    x8[:, 1], a["x8"][b, 1].rearrange("c p n -> p c n"))
            else:
                nc.sync.dma_start(x8, a["x8"][b].rearrange("h c p n -> p h c n"))
            xs8 = xs8_p.tile([128, 2, 2, NQ], f8e4, tag="xs8", name=f"xs8{b}")
            nc.sync.dma_start(xs8, a["xs8"][b].rearrange("h c p n -> p h c n"))
            if first:
                nc.sync.dma_start(wq8, a["wq8"].rearrange("h c p j -> p h c j"))
                nc.sync.dma_start(bqs, a["bq"])
            if stagger:
                nc.sync.dma_start(
                    bt8a[:, 2:5], a["bt8"][2:5].rearrange("h p c q -> p h c q"))
                nc.sync.dma_start(
                    bt8a[:, 5:8], a["bt8"][5:8].rearrange("h p c q -> p h c q"))
                nc.sync.dma_start(wp, a["wpt"].rearrange("c p j -> p c j"))
                nc.sync.dma_start(bps, a["bps"])
            return x8, xs8

        def proj_gen(b, x8, xs8):
            """Yield after each proj psum tile; returns (kt, vt, qt) eagerly."""
            kt = kt_p.tile([128, 4, N], f32r, tag="kt", name=f"kt{b}")
            vt = v_p.tile([128, NCH, 1024], bf16, tag="vt", name=f"vt{b}")
            qt = qt_p.tile([128, 4, NQ], f32r, tag="qt", name=f"qt{b}")
            kterms = ((0, 0), (1, 0), (0, 1))   # (w hi/lo, x hi/lo) pairs;
            # x-lo only needed by term 3, so batch 0's split x8 DMA overlaps

            def emit():
                # kT projection: fp8 hi/lo 3-term DoubleRow, n-major so the
                # first psum tile only needs DMA slice 0. No bias
                # (softmax-invariant). Evac via 2D-strided copies.
                for ns in range(2):             # n slices 0:512, 512:1024
                    n0 = ns * 512
                    for prh in range(2):        # pr pairs (0,1), (2,3)
                        ps = sg_tile()
                        for j in range(2):
                            pr = 2 * prh + j
                            for pi, (wi, xi) in enumerate(kterms):
                                nc.tensor.matmul(
                                    ps[:, j, :],
                                    lhsT=wk8[:, wi, :, pr * 128:(pr + 1) * 128],
                                    rhs=x8[:, xi, :, n0:n0 + 512],
                                    start=(pi == 0), stop=(pi == 2),
                                    perf_mode=PM.DoubleRow,
                                )
                        if (ns + prh) % 2 == 0:
                            nc.vector.tensor_scalar(
                                kt[:, 2 * prh:2 * prh + 2, n0:n0 + 512], ps,
                                K_DESCALE, None, ALU.mult)
                        else:
                            nc.scalar.activation(
                                kt[:, 2 * prh:2 * prh + 2, n0:n0 + 512], ps,
                                AF.Copy, scale=K_DESCALE)
                        yield
                ps = sg_tile()                  # 256-col tails, two prs per tile
                for prh in range(2):
                    for j in range(2):
                        pr = 2 * prh + j
                        c0 = j * 256
                        for pi, (wi, xi) in enumerate(kterms):
                            nc.tensor.matmul(
                                ps[:, prh, c0:c0 + 256],
                                lhsT=wk8[:, wi, :, pr * 128:(pr + 1) * 128],
                                rhs=x8[:, xi, :, 1024:N],
                                start=(pi == 0), stop=(pi == 2),
                                perf_mode=PM.DoubleRow,
                            )
                    nc.scalar.activation(
                        kt[:, 2 * prh:2 * prh + 2, 1024:N],
                        ps[:, prh, :].rearrange("p (g c) -> p g c", g=2),
                        AF.Copy, scale=K_DESCALE)
                yield
                # q projection: fp8 3-term, 2 prs per tile, fused bias on DVE
                for half in range(2):
                    ps = sg_tile()
                    for j in range(2):
                        pr = 2 * half + j
                        for pi, (wi, xi) in enumerate(kterms):
                            nc.tensor.matmul(
                                ps[:, j, :NQ],
                                lhsT=wq8[:, wi, :, pr * 128:(pr + 1) * 128],
                                rhs=xs8[:, xi, :, :],
                                start=(pi == 0), stop=(pi == 2),
                                perf_mode=PM.DoubleRow,
                            )
                    nc.vector.tensor_tensor(
                        qt[:, 2 * half:2 * half + 2, :], ps[:, :, :NQ],
                        bqs[:, 2 * half:2 * half + 2].to_broadcast((128, 2, NQ)),
                        ALU.add,
                    )
                    yield
                # v projection: fp8 hi/lo DoubleRow (K=256 per pass, 3 passes)
                for cn in range(NCH):
                    ps = sg_tile()
                    for hf in range(2):
                        for pi, (xi, wi) in enumerate(((0, 0), (1, 0), (0, 1))):
                            nc.tensor.matmul(
                                ps[:, hf, :],
                                lhsT=x8[:, xi, :, cn * 128:(cn + 1) * 128],
                                rhs=wv8[:, wi, :, hf * 512:(hf + 1) * 512],
                                start=(pi == 0), stop=(pi == 2),
                                perf_mode=PM.DoubleRow,
                            )
                    if cn % 2 == 0:
                        nc.vector.tensor_scalar(
                            vt[:, cn, :], ps.rearrange("p g j -> p (g j)"),
                            V_DESCALE, None, ALU.mult)
                    else:
                        nc.scalar.activation(
                            vt[:, cn, :], ps.rearrange("p g j -> p (g j)"),
                            AF.Copy, scale=V_DESCALE)
                    yield

            return kt, vt, qt, emit()

        def attention(b, h, kt, vt, qt, t2):
            # generator: yields after each score group so the driver can
            # weave projection tiles of the next batch between groups
            yield
            pr, p0 = h // 2, 64 * (h % 2)
            bt8 = bt8a[:, h]
            po = ps_o.tile([128, NQ], f32, tag="po", name=f"po_{b}_{h}")
            acc = None
            e0 = None
            for g in range(NCH // GRP):
                sg = sg_tile()
                for j in range(GRP):
                    c = GRP * g + j
                    nc.tensor.matmul(
                        sg[:, j, :NQ],
                        lhsT=kt[p0:p0 + 64, pr, c * 128:(c + 1) * 128],
                        rhs=qt[p0:p0 + 64, pr, :],
                        start=True, stop=False,
                    )
                    nc.tensor.matmul(
                        sg[:, j, :NQ],
                        lhsT=identp[:, j, 0:2, :],
                        rhs=bt8[:, GRP * g:GRP * (g + 1), :],
                        start=False, stop=True,
                        perf_mode=PM.DoubleRow,
                    )
                e = e_p.tile([128, GRP, NQ], bf16)
                nc.scalar.activation(e, sg[:, :, :NQ], AF.Exp)
                # incremental e-sum accumulation: spreads the adds through
                # the head so the denominator is ready right after the last
                # exp (no end-of-head serial DVE burst blocking the PE SEQ)
                if g == 0:
                    e0 = e
                elif g == 1:
                    acc = esum_p.tile([128, GRP, NQ], bf16, tag="acc")
                    nc.vector.tensor_tensor(acc, e0, e, ALU.add)
                else:
                    nc.vector.tensor_tensor(acc, acc, e, ALU.add)
                    if g == NCH // GRP - 1:
                        # esum8 = fp8(acc * 2^-6): the scale dodges fp8-e4m3's
                        # 240 max (exp sums reach ~8400); reinjected at oraw
                        esum8 = esum_p.tile([128, GRP, NQ], f8e4, tag="esum8")
                        nc.vector.tensor_scalar(
                            esum8, acc, 2.0 ** -6, None, ALU.mult)
                for j in range(GRP):
                    c = GRP * g + j
                    nc.tensor.matmul(
                        po[:, :NQ],
                        lhsT=vt[:, c, h * 128:(h + 1) * 128],
                        rhs=e[:, j, :],
                        start=(c == 0), stop=(c == NCH - 1),
                    )
                yield
            # softmax denominator: the fold ran right after the last in-head
            # add, so this matmul's input is ready with no serial DVE burst
            psm = ps_sum.tile([128, NQ], f32, tag="psm", name=f"psm_{b}_{h}")
            nc.tensor.matmul(psm, lhsT=identp[:, :, 2, :], rhs=esum8,
                             start=True, stop=True, perf_mode=PM.DoubleRow)
            # copy po out of PSUM immediately (decouples the po bank from the
            # recip chain), then normalize on Pool
            oraw = oh_p.tile([128, NQ], bf16, tag="oraw", name=f"oraw_{b}_{h}")
            nc.vector.tensor_scalar(oraw, po, 2.0 ** -6, None, ALU.mult)
            rc = rc_p.tile([128, NQ], bf16)
            nc.vector.reciprocal(rc, psm)
            oh = oh_p.tile([128, NQ], bf16)
            nc.vector.tensor_tensor(oh, oraw, rc, ALU.mult)
            # hswish: t = min(Relu(o + 3 + bv), 6);  th = (o + bv) * t
            # on Pool normally; the last heads of the last batch run on DVE
            # (143ns/op 4x mode) to shorten the serial chain the output
            # projection tail waits on -- nothing queues behind DVE there
            eng = nc.vector if (b == BPC - 1 and h >= 6) else nc.gpsimd
            t1 = t1_p.tile([128, NQ], bf16)
            eng.tensor_scalar(t1, oh, bv2[:, 1, h:h + 1], 0.0, ALU.add, ALU.max)
            eng.tensor_scalar(t1, t1, 6.0, None, ALU.min)
            obv = t1_p.tile([128, NQ], bf16, tag="obv", name=f"obv_{b}_{h}")
            eng.tensor_scalar(obv, oh, bv2[:, 0, h:h + 1], None, ALU.add)
            eng.tensor_tensor(t2[:, h, b % 2, :], obv, t1, ALU.mult)

        def opj_mm(ps, t2, qc, dc, start, stop):
            nc.tensor.matmul(
                ps[:, 0, :OUT],
                lhsT=t2[:, dc, :, :].rearrange(
                    "p bb q -> p (bb q)")[:, qc * 128:(qc + 1) * 128],
                rhs=wp[:, dc, :],
                start=start, stop=stop,
            )

        def out_proj(b, t2, qcs=range(5), deep=False):
            """deep=True (tail only): qc 0,1 read only the PRIOR batch's t2
            half, so their full chains + output DMAs run before the last
            head's t2 even exists; qc 2..4 run dc 0..6 first and the
            just-finished last head's dc 7 afterwards (~6us of PE work
            overlapping the final hswish chain + earlier DMA drain). Holds
            all 3 sg psum bufs, so never weave a deep out_proj into an
            attention phase."""
            def finish(ps, qc):
                ob = ob_p.tile([128, OUT], f32)
                nc.vector.tensor_tensor(ob, ps[:, 0, :OUT], bps, ALU.add)
                nc.sync.dma_start(
                    out_flat[(b - 1) * NQ + qc * 128:(b - 1) * NQ + qc * 128 + 128, :], ob)

            if deep:
                for qc in (0, 1):          # prior-batch columns: no h7 dep
                    ps = sg_tile()
                    for dc in range(8):
                        opj_mm(ps, t2, qc, dc, dc == 0, dc == 7)
                    finish(ps, qc)
                    yield
                held = []
                for qc in (2, 3, 4):
                    ps = sg_tile()
                    for dc in range(7):
                        opj_mm(ps, t2, qc, dc, dc == 0, False)
                    held.append(ps)
                for i, qc in enumerate((2, 3, 4)):
                    opj_mm(held[i], t2, qc, 7, False, True)
                    finish(held[i], qc)
                    yield
                return
            for qc in qcs:
                ps = sg_tile()
                for dc in range(8):
                    opj_mm(ps, t2, qc, dc, dc == 0, dc == 7)
                finish(ps, qc)
                yield

        # prologue: batch 0 inputs + weights in order of first use:
        # kT slices -> q inputs -> v weights -> attention singles
        # x8(0) is the largest first-needed transfer -- it leads the
        # serialized DMA pipe; wk8 (small) follows and lands about when
        # the PE warm bridge ends
        x80, xs80 = dma_x(0, first=True)
        nc.sync.dma_start(wv8, a["wv8"].rearrange("h c p j -> p h c j"))
        nc.sync.dma_start(identp, a["identp"])
        nc.sync.dma_start(
            bt8a[:, 0:2], a["bt8"][0:2].rearrange("h p c q -> p h c q"))
        nc.sync.dma_start(bv2, a["bv2"])

        # warm up the PE p-state during the input-DMA wait: dummy matmuls on
        # a memset scratch (no DMA dependency) bridge the ~3.2us DMA pipe
        # latency continuously, so the first real matmul runs at full clock
        scratch = singles.tile([128, 2, 256], f8e4)
        nc.gpsimd.memset(scratch, 0.0)
        warm = sg_tile()
        for i in range(14):
            nc.tensor.matmul(
                warm[:, 0, :256],
                lhsT=scratch[:, :, 0:128],
                rhs=scratch,
                start=True, stop=True,
                perf_mode=PM.DoubleRow,
            )

        kt, vt, qt, gen0 = proj_gen(0, x80, xs80)
        # drain kT + q + v chunks 0,1 eagerly; attention(0) h0 starts ~6us
        # earlier and weaves the remaining v chunks just-in-time (2/group)
        for _ in range(9):
            next(gen0)
        pending = [gen0]
        opj = None
        t2 = None
        nxt = None
        for b in range(BPC):
            if b % 2 == 0:
                t2 = hs_p.tile([128, H, 2, NQ], bf16, tag="t2", name=f"t2_{b}")
            # interleave remaining proj tiles (this batch's tail + next batch)
            if b + 1 < BPC:
                x8n, xs8n = dma_x(b + 1, stagger=(b == 0))
                nxt = proj_gen(b + 1, x8n, xs8n)
                pending.append(nxt[3])
                if opj is not None:
                    # pair out-proj drains AFTER the projection tiles: by
                    # then the previous batch's last-head t2 is long done,
                    # so the weave never stalls PE on the Pool hswish chain
                    pending.append(opj)
                    opj = None

            for h in range(H):
                for gi, _ in enumerate(attention(b, h, kt, vt, qt, t2)):
                    if gi == 0:
                        ndrain = 0
                    elif b == 0 and h == 0:
                        ndrain = 2
                    elif gi == 5:
                        ndrain = 3
                    else:
                        ndrain = 1
                    for _ in range(ndrain):
                        while pending:
                            if next(pending[0], "done") == "done":
                                pending.pop(0)
                            else:
                                break
            while pending:
                if next(pending[0], "done") == "done":
                    pending.pop(0)
                else:
                    break
            if pending:
                for _ in pending[0]:
                    pass
                pending.pop(0)
            if b % 2 == 1:
                if b + 1 < BPC:
                    opj = out_proj(b, t2)  # interleave with next batch
                else:
                    for _ in out_proj(b, t2, deep=True):
                        pass
            if nxt is not None:
                kt, vt, qt = nxt[0], nxt[1], nxt[2]
                nxt = None


def build():
    import concourse.mybir as mybir
    import concourse.tile as tile
    from concourse import bacc

    nc = bacc.Bacc("TRN2", target_bir_lowering=False, debug=False)
    f32, bf16 = mybir.dt.float32, mybir.dt.bfloat16
    f8e4 = mybir.dt.float8e4
    a = {}

    def din(name, shape, dt=f32):
        a[name] = nc.dram_tensor(name, shape, dt, kind="ExternalInput").ap()

    din("x8", [BPC, 2, 2, 128, N], f8e4)
    din("xs8", [BPC, 2, 2, 128, NQ], f8e4)
    din("wk8", [2, 2, 128, 512], f8e4)
    din("wq8", [2, 2, 128, 512], f8e4)
    din("wv8", [2, 2, 128, 1024], f8e4)
    din("wpt", [8, 128, OUT], bf16)
    din("bq", [128, 4])
    din("bv2", [128, 2, H])
    din("bps", [128, OUT])
    din("bt8", [H, 128, NCH, NQ], f8e4)
    din("identp", [128, 2, 3, 128], f8e4)
    out_ap = nc.dram_tensor("out", [BPC, NQ, OUT], f32, kind="ExternalOutput").ap()

    with tile.TileContext(nc) as tc:
        _body(tc, a, out_ap)
    nc.compile()
    return nc


_NC_CACHE = None


def _get_nc():
    global _NC_CACHE
    if _NC_CACHE is None:
        _NC_CACHE = build()
    return _NC_CACHE


def kernel(**inputs):
    from concourse.bass_utils import run_bass_kernel_spmd

    in_maps = _prep(inputs)
    nc = _get_nc()
    res = run_bass_kernel_spmd(nc, in_maps, list(range(NCORES)))
    out = np.concatenate([res.results[i]["out"] for i in range(NCORES)], axis=0)
    return np.ascontiguousarray(out, dtype=np.float32)


if __name__ == "__main__":
    rng = np.random.default_rng(0)
    print("smoke: building bass module...")
    nc = build()
    print("built ok:", sum(len(bb.instructions) for bb in nc.m.functions[0].blocks), "instructions")



# revision 66
# speedup vs baseline: 1.0012x; 1.0003x over previous
"""AttentionSubsample Trainium2 kernel.

Full (unsharded) inputs in, full output out. Data-parallel over batch:
32 batches -> 8 NeuronCores x 4 batches each. Weights/biases replicated.

Cost-model 192.2us/core (v1 206.4, v0 275.6). PE busy ~165us (86%) --
every GEMM already at its cheapest dtype mode, so the engine-balance
and pipelining below are what the remaining time is made of:
  - ALL projections (kT, q, v) as fp8(e4m3) hi/lo 3-term DoubleRow
    matmuls (0.5 cyc/row, K=256 packed 2 rows/partition): x=xh+xl,
    W=wh+wl, dropping the lo@lo term. The splits are computed on
    PRE-SCALED tensors (SX/SWK/SWQ/SWV) because the raw weights (~0.06)
    put the lo residuals at fp8-e4m3's subnormal floor (2^-9), which
    wrecks the split (k err 0.78% -> 0.12% scaled). Descale folds into
    the kt evacuation (k/(SX*SWQ) so qt*kt is exact against the
    host-scaled q bias) and the v evacuation.
  - k-channel BN bias dropped entirely: softmax over n is invariant to
    per-q shifts and (k+bk)@q shifts every key n equally.
  - score bias added pre-exp on the PE as fp8 DoubleRow identity
    matmuls: lhsT=(I,0)/(0,I) selects one chunk of an adjacent
    bias-chunk pair; bias resident in SBUF, loaded once.
  - softmax denominators: e-tiles accumulated INCREMENTALLY on DVE
    during the head (no end-of-head serial burst blocking the in-order
    PE SEQ), converted to fp8 (x 2^-6, dodging e4m3's 240 max), summed
    by ONE DoubleRow ones-matmul (the ones live in identp plane 2);
    the 2^-6 reinjects via the po evacuation scale.
  - hswish on Pool (tensor_scalar chains); normalize mult + po/out-proj
    evacuations on DVE; kT/v evacuations split ACT/DVE (Pool cannot
    touch PSUM, scalar_tensor_tensor is not a valid Pool opcode on hw).
  - DMA: hi/lo pairs and bias planes merged into single tensors -- each
    DMA costs ~650ns SP + ~625ns HWDGE serialized regardless of size,
    so transfer COUNT dominates startup. Input pools are double-
    buffered so a batch's DMA never queues behind the previous batch's
    reads on the SP SEQ. PE p-state is warmed with DoubleRow matmuls on
    a memset scratch bridging the ~3us DMA pipe latency.
  - software pipelining: batch b+1's projection psum tiles weave
    between batch b's attention score groups (1/group; batch 0's own v
    chunks weave into its first head at 2/group); the pair output
    projection weaves into the NEXT batch after its projection tiles;
    the final pair runs "deep": qc 0,1 (prior-batch columns) fully
    finish + DMA out before the last head's t2 exists, qc 2..4 run
    dc 0..6 first and the last-head dc 7 afterwards, overlapping the
    final hswish chain (whose last heads run on DVE 4x ops, not Pool).
  - PSUM: scores pool 3x[128,2,512] (chunk pairs at bank-aligned 512
    offsets, exp reads the [*, :320] pair in one ACT instr), po + psm
    1 bank each = 8 banks.
  - error budget (hw-measured rel err 1.33e-2 vs 2e-2 gate): fp8-Z
    rounding ~1e-2, projection splits ~1e-3, bf16 e/vt/t2/wpt ~3e-3.
"""

import sys

if "/opt/trn_rl_repo" not in sys.path:
    sys.path.insert(0, "/opt/trn_rl_repo")

import ml_dtypes
import numpy as np

# --- problem constants (hardcoded, must match the grading reference) ---
B, N, C = 32, 1280, 256
H, KD, D = 8, 64, 128          # heads, key dim, value dim per head
NQ = 320                       # subsampled sequence length
OUT = 384
NCORES = 8
BPC = B // NCORES              # batches per core
EPS = 1e-5
NCH = N // 128                 # 10 n-chunks of 128
GRP = 2                        # scores psum group size (n-chunks per group)
SX, SWK, SWQ, SWV = 16.0, 16.0, 128.0, 16.0   # fp8 hi/lo split pre-scales

_SUB_IDX = np.concatenate([
    (np.arange(32)[::2][:, None] * 32 + np.arange(32)[::2][None, :]).reshape(-1),
    1024 + (np.arange(16)[::2][:, None] * 16 + np.arange(16)[::2][None, :]).reshape(-1),
])  # [320] subsample row gather


def _prep(inputs):
    """Host-side: fold BN into weights, reorder channels, shard over cores."""
    f32 = np.float32
    x = np.asarray(inputs["x"], f32)
    g_kv, b_kv = np.asarray(inputs["g_kv"], f32), np.asarray(inputs["b_kv"], f32)
    rm_kv, rv_kv = np.asarray(inputs["rm_kv"], f32), np.asarray(inputs["rv_kv"], f32)
    g_q, b_q = np.asarray(inputs["g_q"], f32), np.asarray(inputs["b_q"], f32)
    rm_q, rv_q = np.asarray(inputs["rm_q"], f32), np.asarray(inputs["rv_q"], f32)
    g_p, b_p = np.asarray(inputs["g_p"], f32), np.asarray(inputs["b_p"], f32)
    rm_p, rv_p = np.asarray(inputs["rm_p"], f32), np.asarray(inputs["rv_p"], f32)
    W_kv = np.asarray(inputs["W_kv"], f32)
    W_q = np.asarray(inputs["W_q"], f32)
    W_p = np.asarray(inputs["W_p"], f32)
    attn_bias = np.asarray(inputs["attn_bias"], f32)
    bias_idxs = np.asarray(inputs["bias_idxs"])

    s_kv = g_kv / np.sqrt(rv_kv + EPS)
    Wkv_f = W_kv * s_kv[:, None]
    bkv_f = b_kv - rm_kv * s_kv
    kidx = np.concatenate([np.arange(h * 192, h * 192 + KD) for h in range(H)])
    vidx = np.concatenate([np.arange(h * 192 + KD, (h + 1) * 192) for h in range(H)])
    wkt = np.ascontiguousarray(Wkv_f[kidx].T).reshape(2, 128, 512)     # [c,128][512 kch]
    wvt = np.ascontiguousarray(Wkv_f[vidx].T).reshape(2, 128, 1024)
    bvd = np.ascontiguousarray(bkv_f[vidx].reshape(8, 128).T)          # [128, H]

    scale = KD ** -0.5
    s_q = g_q / np.sqrt(rv_q + EPS)
    wqt = np.ascontiguousarray((W_q * (s_q * scale)[:, None]).T).reshape(2, 128, 512)
    # qt carries SX*SWQ*(q+bq); kt carries k/(SX*SWQ) -- product is exact
    bq = np.ascontiguousarray(
        ((b_q - rm_q * s_q) * scale).reshape(4, 128).T) * (SX * SWQ)

    # fp8 hi/lo splits are computed on SCALED tensors: the raw weights
    # (~0.06, and wqt ~0.008 with the attention scale folded) put the lo
    # residuals in fp8-e4m3's subnormal range (floor 2^-9), wrecking the
    # split accuracy (k err 0.78% -> 0.12% with scaling). The product
    # scales are folded into the kt/v evacuations and the q bias.
    f8_ = ml_dtypes.float8_e4m3

    def split8(arr, s):
        a = arr * s
        h = a.astype(f8_)
        l = (a - h.astype(np.float32)).astype(f8_)
        return h, l

    wk8h, wk8l = split8(wkt, SWK)
    wq8h, wq8l = split8(wqt, SWQ)

    s_p = g_p / np.sqrt(rv_p + EPS)
    wpt = np.ascontiguousarray((W_p * s_p[:, None]).T / 6.0).reshape(
        8, 128, OUT).astype(ml_dtypes.bfloat16)
    bps = np.ascontiguousarray(np.broadcast_to(b_p - rm_p * s_p, (128, OUT))).astype(np.float32)

    biasT = attn_bias[:, bias_idxs].transpose(0, 2, 1)                 # [H, N, NQ]
    bias_cpq = biasT.reshape(H, NCH, 128, NQ).transpose(0, 2, 1, 3)    # [H,128,NCH,NQ]
    f8 = ml_dtypes.float8_e4m3
    # bias fp8, stored once per head; the DoubleRow identity pair (I,0)/(0,I)
    # selects one chunk of an adjacent pair per instruction
    bt8 = np.ascontiguousarray(bias_cpq).astype(f8)                    # [H,128,NCH,NQ]

    identp = np.zeros((128, 2, 3, 128), f8)
    identp[np.arange(128), 0, 0, np.arange(128)] = 1.0
    identp[np.arange(128), 1, 1, np.arange(128)] = 1.0
    identp[:, :, 2, :] = 1.0          # ones plane for the fp8 sums matmul

    wv8h, wv8l = split8(wvt, SWV)

    # hi/lo pairs merged into single tensors: each DMA costs ~625ns of
    # serialized HWDGE time regardless of size, so fewer+larger transfers
    # dominate the startup latency
    wk8 = np.stack([wk8h, wk8l])               # [2(hl), 2, 128, 512]
    wq8 = np.stack([wq8h, wq8l])
    wv8 = np.stack([wv8h, wv8l])               # [2(hl), 2, 128, 1024]
    bv2 = np.stack([bvd, bvd + 3.0], axis=1).astype(np.float32)  # [128, 2, H]

    xs = x[:, _SUB_IDX, :]                                             # [B, NQ, C]
    in_maps = []
    for i in range(NCORES):
        sl = slice(i * BPC, (i + 1) * BPC)
        xt = np.ascontiguousarray(x[sl].transpose(0, 2, 1)).reshape(BPC, 2, 128, N)
        x8h, x8l = split8(xt, SX)
        xst = np.ascontiguousarray(xs[sl].transpose(0, 2, 1)).reshape(BPC, 2, 128, NQ)
        xs8h, xs8l = split8(xst, SX)
        in_maps.append({
            "x8": np.ascontiguousarray(np.stack([x8h, x8l], axis=1)),
            "xs8": np.ascontiguousarray(np.stack([xs8h, xs8l], axis=1)),
            "wv8": wv8, "wk8": wk8, "wq8": wq8,
            "wpt": wpt,
            "bq": bq, "bv2": bv2, "bps": bps,
            "bt8": bt8,
            "identp": identp,
        })
    return in_maps


def _body(tc, a, out_ap):
    import concourse.bass as bass  # noqa: F401
    import concourse.mybir as mybir
    from contextlib import ExitStack

    nc = tc.nc
    f32 = mybir.dt.float32
    f32r = mybir.dt.float32r
    bf16 = mybir.dt.bfloat16
    f8e4 = mybir.dt.float8e4
    AF = mybir.ActivationFunctionType
    ALU = mybir.AluOpType
    PM = mybir.MatmulPerfMode
    # kt carries k/(SX*SWQ) so qt (SX*SWQ*(q+bq)) times kt is exact;
    # v psum carries SX*SWV*v
    K_DESCALE = 1.0 / (SX * SWK * SX * SWQ)
    V_DESCALE = 1.0 / (SX * SWV)

    with ExitStack() as ctx:
        ctx.enter_context(
            nc.allow_low_precision(reason="bf16 o-side + fp8 bias matmuls are deliberate; verified vs fp32 reference")
        )
        singles = ctx.enter_context(tc.tile_pool(name="singles", bufs=1))
        # DMA order matters at startup: first-needed weights first (wk8 ->
        # kT projection of batch 0), small attention-phase tiles later.
        # hi/lo fp8 pairs live in one tile (dim1 = hi/lo) = one DMA each.
        wk8 = singles.tile([128, 2, 2, 512], f8e4)
        nc.sync.dma_start(wk8, a["wk8"].rearrange("h c p j -> p h c j"))
        wq8 = singles.tile([128, 2, 2, 512], f8e4)
        bqs = singles.tile([128, 4], f32)
        wv8 = singles.tile([128, 2, 2, 1024], f8e4)
        wp = singles.tile([128, 8, OUT], bf16)
        bv2 = singles.tile([128, 2, H], f32)
        identp = singles.tile([128, 2, 3, 128], f8e4)
        bps = singles.tile([128, OUT], f32)
        bt8a = singles.tile([128, H, NCH, NQ], f8e4)

        # bufs=2: batch b+1's input DMA must not wait on batch b's projection
        # reads -- a bufs=1 ring would stall the DMA on the SP SEQ, blocking
        # every later-issued DMA (bt8, wp) behind it
        x8_p = ctx.enter_context(tc.tile_pool(name="x8", bufs=2))
        xs8_p = ctx.enter_context(tc.tile_pool(name="xs8", bufs=2))
        kt_p = ctx.enter_context(tc.tile_pool(name="kt", bufs=2))
        v_p = ctx.enter_context(tc.tile_pool(name="v", bufs=2))
        qt_p = ctx.enter_context(tc.tile_pool(name="qt", bufs=3))
        e_p = ctx.enter_context(tc.tile_pool(name="e", bufs=12))
        esum_p = ctx.enter_context(tc.tile_pool(name="esum", bufs=3))
        rc_p = ctx.enter_context(tc.tile_pool(name="rc", bufs=3))
        oh_p = ctx.enter_context(tc.tile_pool(name="oh", bufs=3))
        t1_p = ctx.enter_context(tc.tile_pool(name="t1", bufs=3))
        hs_p = ctx.enter_context(tc.tile_pool(name="hs", bufs=2))
        ob_p = ctx.enter_context(tc.tile_pool(name="ob", bufs=4))
        ps_sg = ctx.enter_context(tc.tile_pool(name="ps_sg", bufs=3, space="PSUM"))
        ps_o = ctx.enter_context(tc.tile_pool(name="ps_o", bufs=1, space="PSUM"))
        ps_sum = ctx.enter_context(tc.tile_pool(name="ps_sum", bufs=1, space="PSUM"))

        _wt_n = [0]

        def sg_tile():
            _wt_n[0] += 1
            return ps_sg.tile([128, GRP, 512], f32, tag="sg", name=f"sg{_wt_n[0]}")

        out_flat = out_ap.rearrange("b q o -> (b q) o")

        def dma_x(b, first=False, stagger=False):
            """Issue input DMAs for batch b; returns (x8, xs8).

            x8 data goes FIRST: the weave projection of batch b starts
            consuming it within ~2us of emission, while the staggered
            attention-phase singles (bt8, wp) are not needed until later.
            """
            x8 = x8_p.tile([128, 2, 2, N], f8e4, tag="x8", name=f"x8{b}")
            if first:
                nc.sync.dma_start(
                    x8[:, 0], a["x8"][b, 0].rearrange("c p n -> p c n"))
                nc.sync.dma_start(
                    x8[:, 1], a["x8"][b, 1].rearrange("c p n -> p c n"))
            else:
                nc.sync.dma_start(x8, a["x8"][b].rearrange("h c p n -> p h c n"))
            xs8 = xs8_p.tile([128, 2, 2, NQ], f8e4, tag="xs8", name=f"xs8{b}")
            nc.sync.dma_start(xs8, a["xs8"][b].rearrange("h c p n -> p h c n"))
            if first:
                nc.sync.dma_start(wq8, a["wq8"].rearrange("h c p j -> p h c j"))
                nc.sync.dma_start(bqs, a["bq"])
            if stagger:
                nc.sync.dma_start(
                    bt8a[:, 2:5], a["bt8"][2:5].rearrange("h p c q -> p h c q"))
                nc.sync.dma_start(
                    bt8a[:, 5:8], a["bt8"][5:8].rearrange("h p c q -> p h c q"))
                nc.sync.dma_start(wp, a["wpt"].rearrange("c p j -> p c j"))
                nc.sync.dma_start(bps, a["bps"])
            return x8, xs8

        def proj_gen(b, x8, xs8):
            """Yield after each proj psum tile; returns (kt, vt, qt) eagerly."""
            kt = kt_p.tile([128, 4, N], f32r, tag="kt", name=f"kt{b}")
            vt = v_p.tile([128, NCH, 1024], bf16, tag="vt", name=f"vt{b}")
            qt = qt_p.tile([128, 4, NQ], f32r, tag="qt", name=f"qt{b}")
            kterms = ((0, 0), (1, 0), (0, 1))   # (w hi/lo, x hi/lo) pairs;
            # x-lo only needed by term 3, so batch 0's split x8 DMA overlaps

            def emit():
                # kT projection: fp8 hi/lo 3-term DoubleRow, n-major so the
                # first psum tile only needs DMA slice 0. No bias
                # (softmax-invariant). Evac via 2D-strided copies.
                for ns in range(2):             # n slices 0:512, 512:1024
                    n0 = ns * 512
                    for prh in range(2):        # pr pairs (0,1), (2,3)
                        ps = sg_tile()
                        for j in range(2):
                            pr = 2 * prh + j
                            for pi, (wi, xi) in enumerate(kterms):
                                nc.tensor.matmul(
                                    ps[:, j, :],
                                    lhsT=wk8[:, wi, :, pr * 128:(pr + 1) * 128],
                                    rhs=x8[:, xi, :, n0:n0 + 512],
                                    start=(pi == 0), stop=(pi == 2),
                                    perf_mode=PM.DoubleRow,
                                )
                        if (ns + prh) % 2 == 0:
                            nc.vector.tensor_scalar(
                                kt[:, 2 * prh:2 * prh + 2, n0:n0 + 512], ps,
                                K_DESCALE, None, ALU.mult)
                        else:
                            nc.scalar.activation(
                                kt[:, 2 * prh:2 * prh + 2, n0:n0 + 512], ps,
                                AF.Copy, scale=K_DESCALE)
                        yield
                ps = sg_tile()                  # 256-col tails, two prs per tile
                for prh in range(2):
                    for j in range(2):
                        pr = 2 * prh + j
                        c0 = j * 256
                        for pi, (wi, xi) in enumerate(kterms):
                            nc.tensor.matmul(
                                ps[:, prh, c0:c0 + 256],
                                lhsT=wk8[:, wi, :, pr * 128:(pr + 1) * 128],
                                rhs=x8[:, xi, :, 1024:N],
                                start=(pi == 0), stop=(pi == 2),
                                perf_mode=PM.DoubleRow,
                            )
                    nc.scalar.activation(
                        kt[:, 2 * prh:2 * prh + 2, 1024:N],
                        ps[:, prh, :].rearrange("p (g c) -> p g c", g=2),
                        AF.Copy, scale=K_DESCALE)
                yield
                # q projection: fp8 3-term, 2 prs per tile, fused bias on DVE
                for half in range(2):
                    ps = sg_tile()
                    for j in range(2):
                        pr = 2 * half + j
                        for pi, (wi, xi) in enumerate(kterms):
                            nc.tensor.matmul(
                                ps[:, j, :NQ],
                                lhsT=wq8[:, wi, :, pr * 128:(pr + 1) * 128],
                                rhs=xs8[:, xi, :, :],
                                start=(pi == 0), stop=(pi == 2),
                                perf_mode=PM.DoubleRow,
                            )
                    nc.vector.tensor_tensor(
                        qt[:, 2 * half:2 * half + 2, :], ps[:, :, :NQ],
                        bqs[:, 2 * half:2 * half + 2].to_broadcast((128, 2, NQ)),
                        ALU.add,
                    )
                    yield
                # v projection: fp8 hi/lo DoubleRow (K=256 per pass, 3 passes)
                for cn in range(NCH):
                    ps = sg_tile()
                    for hf in range(2):
                        for pi, (xi, wi) in enumerate(((0, 0), (1, 0), (0, 1))):
                            nc.tensor.matmul(
                                ps[:, hf, :],
                                lhsT=x8[:, xi, :, cn * 128:(cn + 1) * 128],
                                rhs=wv8[:, wi, :, hf * 512:(hf + 1) * 512],
                                start=(pi == 0), stop=(pi == 2),
                                perf_mode=PM.DoubleRow,
                            )
                    if cn % 2 == 0:
                        nc.vector.tensor_scalar(
                            vt[:, cn, :], ps.rearrange("p g j -> p (g j)"),
                            V_DESCALE, None, ALU.mult)
                    else:
                        nc.scalar.activation(
                            vt[:, cn, :], ps.rearrange("p g j -> p (g j)"),
                            AF.Copy, scale=V_DESCALE)
                    yield

            return kt, vt, qt, emit()

        def attention(b, h, kt, vt, qt, t2):
            # generator: yields after each score group so the driver can
            # weave projection tiles of the next batch between groups
            yield
            pr, p0 = h // 2, 64 * (h % 2)
            bt8 = bt8a[:, h]
            po = ps_o.tile([128, NQ], f32, tag="po", name=f"po_{b}_{h}")
            acc = None
            e0 = None
            for g in range(NCH // GRP):
                sg = sg_tile()
                for j in range(GRP):
                    c = GRP * g + j
                    nc.tensor.matmul(
                        sg[:, j, :NQ],
                        lhsT=kt[p0:p0 + 64, pr, c * 128:(c + 1) * 128],
                        rhs=qt[p0:p0 + 64, pr, :],
                        start=True, stop=False,
                    )
                    nc.tensor.matmul(
                        sg[:, j, :NQ],
                        lhsT=identp[:, j, 0:2, :],
                        rhs=bt8[:, GRP * g:GRP * (g + 1), :],
                        start=False, stop=True,
                        perf_mode=PM.DoubleRow,
                    )
                e = e_p.tile([128, GRP, NQ], bf16)
                nc.scalar.activation(e, sg[:, :, :NQ], AF.Exp)
                # incremental e-sum accumulation: spreads the adds through
                # the head so the denominator is ready right after the last
                # exp (no end-of-head serial DVE burst blocking the PE SEQ)
                if g == 0:
                    e0 = e
                elif g == 1:
                    acc = esum_p.tile([128, GRP, NQ], bf16, tag="acc")
                    nc.vector.tensor_tensor(acc, e0, e, ALU.add)
                else:
                    nc.vector.tensor_tensor(acc, acc, e, ALU.add)
                    if g == NCH // GRP - 1:
                        # esum8 = fp8(acc * 2^-6): the scale dodges fp8-e4m3's
                        # 240 max (exp sums reach ~8400); reinjected at oraw
                        esum8 = esum_p.tile([128, GRP, NQ], f8e4, tag="esum8")
                        nc.vector.tensor_scalar(
                            esum8, acc, 2.0 ** -6, None, ALU.mult)
                for j in range(GRP):
                    c = GRP * g + j
                    nc.tensor.matmul(
                        po[:, :NQ],
                        lhsT=vt[:, c, h * 128:(h + 1) * 128],
                        rhs=e[:, j, :],
                        start=(c == 0), stop=(c == NCH - 1),
                    )
                yield
            # softmax denominator: the fold ran right after the last in-head
            # add, so this matmul's input is ready with no serial DVE burst
            psm = ps_sum.tile([128, NQ], f32, tag="psm", name=f"psm_{b}_{h}")
            nc.tensor.matmul(psm, lhsT=identp[:, :, 2, :], rhs=esum8,
                             start=True, stop=True, perf_mode=PM.DoubleRow)
            # copy po out of PSUM immediately (decouples the po bank from the
            # recip chain), then normalize on Pool
            oraw = oh_p.tile([128, NQ], bf16, tag="oraw", name=f"oraw_{b}_{h}")
            nc.vector.tensor_scalar(oraw, po, 2.0 ** -6, None, ALU.mult)
            rc = rc_p.tile([128, NQ], bf16)
            nc.vector.reciprocal(rc, psm)
            oh = oh_p.tile([128, NQ], bf16)
            nc.vector.tensor_tensor(oh, oraw, rc, ALU.mult)
            # hswish: t = min(Relu(o + 3 + bv), 6);  th = (o + bv) * t
            # on Pool normally; the last heads of the last batch run on DVE
            # (143ns/op 4x mode) to shorten the serial chain the output
            # projection tail waits on -- nothing queues behind DVE there
            eng = nc.vector if (b == BPC - 1 and h >= 6) else nc.gpsimd
            t1 = t1_p.tile([128, NQ], bf16)
            eng.tensor_scalar(t1, oh, bv2[:, 1, h:h + 1], 0.0, ALU.add, ALU.max)
            eng.tensor_scalar(t1, t1, 6.0, None, ALU.min)
            obv = t1_p.tile([128, NQ], bf16, tag="obv", name=f"obv_{b}_{h}")
            eng.tensor_scalar(obv, oh, bv2[:, 0, h:h + 1], None, ALU.add)
            eng.tensor_tensor(t2[:, h, b % 2, :], obv, t1, ALU.mult)

        def opj_mm(ps, t2, qc, dc, start, stop):
            nc.tensor.matmul(
                ps[:, 0, :OUT],
                lhsT=t2[:, dc, :, :].rearrange(
                    "p bb q -> p (bb q)")[:, qc * 128:(qc + 1) * 128],
                rhs=wp[:, dc, :],
                start=start, stop=stop,
            )

        def out_proj(b, t2, qcs=range(5), deep=False):
            """deep=True (tail only): qc 0,1 read only the PRIOR batch's t2
            half, so their full chains + output DMAs run before the last
            head's t2 even exists; qc 2..4 run dc 0..6 first and the
            just-finished last head's dc 7 afterwards (~6us of PE work
            overlapping the final hswish chain + earlier DMA drain). Holds
            all 3 sg psum bufs, so never weave a deep out_proj into an
            attention phase."""
            def finish(ps, qc):
                ob = ob_p.tile([128, OUT], f32)
                nc.vector.tensor_tensor(ob, ps[:, 0, :OUT], bps, ALU.add)
                nc.sync.dma_start(
                    out_flat[(b - 1) * NQ + qc * 128:(b - 1) * NQ + qc * 128 + 128, :], ob)

            if deep:
                for qc in (0, 1):          # prior-batch columns: no h7 dep
                    ps = sg_tile()
                    for dc in range(8):
                        opj_mm(ps, t2, qc, dc, dc == 0, dc == 7)
                    finish(ps, qc)
                    yield
                held = []
                for qc in (2, 3, 4):
                    ps = sg_tile()
                    for dc in range(7):
                        opj_mm(ps, t2, qc, dc, dc == 0, False)
                    held.append(ps)
                for i, qc in enumerate((2, 3, 4)):
                    opj_mm(held[i], t2, qc, 7, False, True)
                    finish(held[i], qc)
                    yield
                return
            for qc in qcs:
                ps = sg_tile()
                for dc in range(8):
                    opj_mm(ps, t2, qc, dc, dc == 0, dc == 7)
                finish(ps, qc)
                yield

        # prologue: batch 0 inputs + weights in order of first use:
        # kT slices -> q inputs -> v weights -> attention singles
        # x8(0) is the largest first-needed transfer -- it leads the
        # serialized DMA pipe; wk8 (small) follows and lands about when
        # the PE warm bridge ends
        x80, xs80 = dma_x(0, first=True)
        nc.sync.dma_start(wv8, a["wv8"].rearrange("h c p j -> p h c j"))
        nc.sync.dma_start(identp, a["identp"])
        nc.sync.dma_start(
            bt8a[:, 0:1], a["bt8"][0:1].rearrange("h p c q -> p h c q"))
        nc.sync.dma_start(bv2, a["bv2"])
        nc.sync.dma_start(
            bt8a[:, 1:2], a["bt8"][1:2].rearrange("h p c q -> p h c q"))

        # warm up the PE p-state during the input-DMA wait: dummy matmuls on
        # a memset scratch (no DMA dependency) bridge the ~3.2us DMA pipe
        # latency continuously, so the first real matmul runs at full clock
        scratch = singles.tile([128, 2, 256], f8e4)
        nc.gpsimd.memset(scratch, 0.0)
        warm = sg_tile()
        for i in range(14):
            nc.tensor.matmul(
                warm[:, 0, :256],
                lhsT=scratch[:, :, 0:128],
                rhs=scratch,
                start=True, stop=True,
                perf_mode=PM.DoubleRow,
            )

        kt, vt, qt, gen0 = proj_gen(0, x80, xs80)
        # drain kT + q + v chunks 0,1 eagerly; attention(0) h0 starts ~6us
        # earlier and weaves the remaining v chunks just-in-time (2/group)
        for _ in range(9):
            next(gen0)
        pending = [gen0]
        opj = None
        t2 = None
        nxt = None
        for b in range(BPC):
            if b % 2 == 0:
                t2 = hs_p.tile([128, H, 2, NQ], bf16, tag="t2", name=f"t2_{b}")
            # interleave remaining proj tiles (this batch's tail + next batch)
            if b + 1 < BPC:
                x8n, xs8n = dma_x(b + 1, stagger=(b == 0))
                nxt = proj_gen(b + 1, x8n, xs8n)
                pending.append(nxt[3])
                if opj is not None:
                    # pair out-proj drains AFTER the projection tiles: by
                    # then the previous batch's last-head t2 is long done,
                    # so the weave never stalls PE on the Pool hswish chain
                    pending.append(opj)
                    opj = None

            for h in range(H):
                for gi, _ in enumerate(attention(b, h, kt, vt, qt, t2)):
                    if gi == 0:
                        ndrain = 0
                    elif b == 0 and h == 0:
                        ndrain = 2
                    elif gi == 5:
                        ndrain = 3
                    else:
                        ndrain = 1
                    for _ in range(ndrain):
                        while pending:
                            if next(pending[0], "done") == "done":
                                pending.pop(0)
                            else:
                                break
            while pending:
                if next(pending[0], "done") == "done":
                    pending.pop(0)
                else:
                    break
            if pending:
                for _ in pending[0]:
                    pass
                pending.pop(0)
            if b % 2 == 1:
                if b + 1 < BPC:
                    opj = out_proj(b, t2)  # interleave with next batch
                else:
                    for _ in out_proj(b, t2, deep=True):
                        pass
            if nxt is not None:
                kt, vt, qt = nxt[0], nxt[1], nxt[2]
                nxt = None


def build():
    import concourse.mybir as mybir
    import concourse.tile as tile
    from concourse import bacc

    nc = bacc.Bacc("TRN2", target_bir_lowering=False, debug=False)
    f32, bf16 = mybir.dt.float32, mybir.dt.bfloat16
    f8e4 = mybir.dt.float8e4
    a = {}

    def din(name, shape, dt=f32):
        a[name] = nc.dram_tensor(name, shape, dt, kind="ExternalInput").ap()

    din("x8", [BPC, 2, 2, 128, N], f8e4)
    din("xs8", [BPC, 2, 2, 128, NQ], f8e4)
    din("wk8", [2, 2, 128, 512], f8e4)
    din("wq8", [2, 2, 128, 512], f8e4)
    din("wv8", [2, 2, 128, 1024], f8e4)
    din("wpt", [8, 128, OUT], bf16)
    din("bq", [128, 4])
    din("bv2", [128, 2, H])
    din("bps", [128, OUT])
    din("bt8", [H, 128, NCH, NQ], f8e4)
    din("identp", [128, 2, 3, 128], f8e4)
    out_ap = nc.dram_tensor("out", [BPC, NQ, OUT], f32, kind="ExternalOutput").ap()

    with tile.TileContext(nc) as tc:
        _body(tc, a, out_ap)
    nc.compile()
    return nc


_NC_CACHE = None


def _get_nc():
    global _NC_CACHE
    if _NC_CACHE is None:
        _NC_CACHE = build()
    return _NC_CACHE


def kernel(**inputs):
    from concourse.bass_utils import run_bass_kernel_spmd

    in_maps = _prep(inputs)
    nc = _get_nc()
    res = run_bass_kernel_spmd(nc, in_maps, list(range(NCORES)))
    out = np.concatenate([res.results[i]["out"] for i in range(NCORES)], axis=0)
    return np.ascontiguousarray(out, dtype=np.float32)


if __name__ == "__main__":
    rng = np.random.default_rng(0)
    print("smoke: building bass module...")
    nc = build()
    print("built ok:", sum(len(bb.instructions) for bb in nc.m.functions[0].blocks), "instructions")

